# revision 1
# baseline (speedup 1.0000x reference)
"""Trainium2 Bass kernel for nn_CrossAttentionBlock (B=4, T=4096, C=512, H=8,
INNER=2048, NIN=2) on 8 NeuronCores.

Sharding: core c handles batch b=c//2, token half h=c%2 (2048 tokens each).
All per-token math is local; the only cross-core coupling is the linear-
attention context (ctx = k^T v, [H,64,64] per batch) and k_sum, reduced with
pair-wise AllReduces (cores 2b and 2b+1).

On-chip layout: the residual stream and all dense math are feature-major
([128 features, 512 tokens] fp32r tiles) so every projection/FFN matmul runs
with a 512-wide moving dim at full PE rate. k/v are produced token-major for
the ctx contraction. LN stats and partition-broadcasts are done with small
ones/selector matmuls on the PE. The softmax-q normalization and the
linear-attention D^-1 are folded into one reciprocal + broadcast pass using
unnormalized E = exp(qp):  out = E/S + sum_i (E @ ctx_i) / G_i with
G_i = sum_d E * ksum_i (the 1e-8 eps is ~1e-6 relative here and dropped).
"""
import os
import numpy as np

import concourse.bass as bass
import concourse.tile as tile
from concourse import mybir
from concourse.vector_clock import ScopedClock
from concourse.bass_utils import run_bass_kernel_spmd

F32 = mybir.dt.float32
F32R = mybir.dt.float32r
AF = mybir.ActivationFunctionType
OP = mybir.AluOpType

B, T, C, H, D, INNER, NIN = 4, 4096, 512, 8, 64, 2048, 2
N_CORES = 8
NTOK = 2048          # tokens per core
CHUNK = 512          # tokens per chunk
NCH = NTOK // CHUNK  # 4 chunks
FT = C // 128        # 4 feature tiles
IT = INNER // 128    # 16 inner tiles
LN_EPS = 1e-5
GROUPS = [[0, 1], [2, 3], [4, 5], [6, 7]]

_split_counter = [0]


def _split_multi_waits(nc):
    """This walrus build only supports one sync-wait per instruction; move
    extra waits onto same-engine NoOps placed immediately before."""
    for f in nc.m.functions:
        for blk in f.blocks:
            out = []
            changed = False
            for inst in blk.instructions:
                si = inst.sync_info
                if si is not None and si.on_wait and len(si.on_wait) > 1:
                    waits = list(si.on_wait)
                    for w in waits[:-1]:
                        _split_counter[0] += 1
                        nop = mybir.InstNoOp(
                            name=f"I-waitsplit-{_split_counter[0]}", ins=[], outs=[]
                        )
                        nop.engine = inst.engine
                        nop.sync_info = mybir.SyncInfo(on_wait=[w], on_update=[])
                        out.append(nop)
                    si.on_wait = waits[-1:]
                    inst.sync_info = si
                    changed = True
                out.append(inst)
            if changed:
                blk.instructions = out


class _TC(tile.TileContext):
    def _drain_and_barrier(self, tick_clock, wait_clock):
        drain_inst = self.nc.sync.drain()
        wait_clock.add_sem_waits(
            drain_inst.ins, ScopedClock({None: tick_clock.global_clock})
        )
        si = drain_inst.ins.sync_info
        if si is not None and si.on_wait and len(si.on_wait) > 1:
            waits = list(si.on_wait)
            si.on_wait = waits[:1]
            drain_inst.ins.sync_info = si
            for i in range(1, len(waits)):
                extra = self.nc.sync.drain()
                esi = extra.ins.sync_info
                if esi is None:
                    extra.ins.sync_info = mybir.SyncInfo(
                        on_wait=waits[i : i + 1], on_update=[]
                    )
                else:
                    esi.on_wait = waits[i : i + 1]
                    extra.ins.sync_info = esi
        self.nc.all_engine_barrier()
        assert self.sems is not None
        popped = self.nc._tile_sem_poison_stack.pop()
        assert popped is self._sem_poison
        self.nc.clear_and_free_semaphores(list(self.sems.allocated().values()))
        self.nc.all_engine_barrier()


def _build_program(split=True):
    nc = bass.Bass("TRN2", target_bir_lowering=False, debug=False, num_devices=N_CORES)
    I = {}

    def di(name, shape):
        I[name] = nc.dram_tensor(name, list(shape), F32, kind="ExternalInput").ap()

    di("xT", [C, NTOK])
    di("ysT", [NIN, C, NTOK])
    for w in ["wq", "wo", "saq", "sak", "sav", "sao"]:
        di(w, [C, C])
    di("wk", [NIN, C, C])
    di("wv", [NIN, C, C])
    di("f1w1", [C, INNER])
    di("f1w2", [INNER, C])
    di("f2w1", [C, INNER])
    di("f2w2", [INNER, C])
    for bname in ["bq_c", "bo_c", "saq_c", "sao_c", "f1b2_c", "f2b2_c"]:
        di(bname, [128, FT])
    di("f1b1_c", [128, IT])
    di("f2b1_c", [128, IT])
    di("bk_r", [NIN, 1, C])
    di("bv_r", [NIN, 1, C])
    di("sak_r", [1, C])
    di("sav_r", [1, C])
    for lname in ["ln1", "ln3", "ln4", "ln5"]:
        di(lname + "_g", [128, FT])
        di(lname + "_b", [128, FT])
    di("ln2_g", [NIN, 128, FT])
    di("ln2_b", [NIN, 128, FT])
    di("ones_c", [1, 128])
    di("ones_r", [128, 1])
    di("sgbase", [FT, 128, 24])
    di("sel8", [FT, 8, 128])
    di("zz", [128, 128])

    out_t = nc.dram_tensor("outT", [C, NTOK], F32, kind="ExternalOutput").ap()

    with _TC(nc) as tc:
        _Emitter(nc, tc, I, out_t).run()
    if split:
        _split_multi_waits(nc)
    return nc


class _Emitter:
    def __init__(self, nc, tc, I, out_t):
        self.nc, self.tc, self.I, self.out_t = nc, tc, I, out_t

    # ---------------- helpers ----------------
    def layer_norm(self, x_tiles, gt, bt):
        nc = self.nc
        sum_ps = self.p_stats.tile([1, CHUNK], F32, tag="stats", name="stats")
        for k in range(FT):
            nc.tensor.matmul(sum_ps, self.ONESR, x_tiles[k],
                             start=(k == 0), stop=(k == FT - 1))
        srow = self.rows.tile([1, CHUNK], F32, tag="rows", name="rows")
        nc.vector.tensor_copy(srow, sum_ps)
        xsq = []
        for k in range(FT):
            sq = self.lntmp.tile([128, CHUNK], F32R, tag="xsq", name="xsq")
            nc.scalar.activation(out=sq, in_=x_tiles[k].bitcast(F32),
                                 func=AF.Square)
            xsq.append(sq)
        sq_ps = self.p_stats.tile([1, CHUNK], F32, tag="stats", name="stats")
        for k in range(FT):
            nc.tensor.matmul(sq_ps, self.ONESR, xsq[k],
                             start=(k == 0), stop=(k == FT - 1))
        qrow = self.rows.tile([1, CHUNK], F32, tag="rows", name="rows")
        nc.vector.tensor_copy(qrow, sq_ps)
        mrow = self.rows.tile([1, CHUNK], F32, tag="rows", name="rows")
        nc.vector.tensor_scalar(out=mrow, in0=srow, scalar1=1.0 / C,
                                scalar2=None, op0=OP.mult)
        m2 = self.rows.tile([1, CHUNK], F32, tag="rows", name="rows")
        nc.vector.tensor_tensor(out=m2, in0=mrow, in1=mrow, op=OP.mult)
        v1 = self.rows.tile([1, CHUNK], F32, tag="rows", name="rows")
        nc.vector.tensor_scalar(out=v1, in0=qrow, scalar1=1.0 / C,
                                scalar2=None, op0=OP.mult)
        var = self.rows.tile([1, CHUNK], F32, tag="rows", name="rows")
        nc.vector.tensor_tensor(out=var, in0=v1, in1=m2, op=OP.subtract)
        sq_ = self.rows.tile([1, CHUNK], F32, tag="rows", name="rows")
        nc.scalar.activation(out=sq_, in_=var, func=AF.Sqrt, bias=self.EPS,
                             scale=1.0)
        arow = self.rows.tile([1, CHUNK], F32R, tag="rows", name="rows")
        with nc.allow_low_precision(reason="fp32r feeds matmul"):
            nc.vector.reciprocal(out=arow, in_=sq_)
        negm = self.rows.tile([1, CHUNK], F32, tag="rows", name="rows")
        nc.vector.tensor_scalar(out=negm, in0=srow, scalar1=-1.0 / C,
                                scalar2=None, op0=OP.mult)
        brow = self.rows.tile([1, CHUNK], F32R, tag="rows", name="rows")
        with nc.allow_low_precision(reason="fp32r feeds matmul"):
            nc.vector.tensor_tensor(out=brow, in0=negm, in1=arow.bitcast(F32),
                                    op=OP.mult)
        a_ps = self.p_bc.tile([128, CHUNK], F32, tag="bc", name="bc")
        nc.tensor.matmul(a_ps, self.ONESC, arow, start=True, stop=True)
        b_ps = self.p_bc.tile([128, CHUNK], F32, tag="bc", name="bc")
        nc.tensor.matmul(b_ps, self.ONESC, brow, start=True, stop=True)
        bsb = self.lntmp.tile([128, CHUNK], F32, tag="bsb", name="bsb")
        nc.scalar.activation(out=bsb, in_=b_ps, func=AF.Copy, bias=0.0,
                             scale=1.0)
        asb = self.lntmp.tile([128, CHUNK], F32, tag="asb", name="asb")
        nc.scalar.activation(out=asb, in_=a_ps, func=AF.Copy, bias=0.0,
                             scale=1.0)
        outs = []
        for k in range(FT):
            t1 = self.lntmp.tile([128, CHUNK], F32, tag="lnt", name="lnt")
            nc.vector.tensor_tensor(out=t1, in0=x_tiles[k].bitcast(F32),
                                    in1=asb, op=OP.mult)
            t2 = self.lntmp.tile([128, CHUNK], F32, tag="lnt", name="lnt")
            nc.vector.tensor_tensor(out=t2, in0=t1, in1=bsb, op=OP.add)
            xk = self.xnp.tile([128, CHUNK], F32R, tag="xn", name="xn")
            nc.scalar.activation(out=xk, in_=t2, func=AF.Identity,
                                 bias=bt[:, k : k + 1], scale=gt[:, k : k + 1])
            outs.append(xk)
        return outs

    def proj_fm_psum(self, w_tiles, xn_tiles, m):
        ps = self.p_mm.tile([128, CHUNK], F32, tag="mm", name="mm")
        for k in range(FT):
            self.nc.tensor.matmul(ps, w_tiles[k][:, 128 * m : 128 * (m + 1)],
                                  xn_tiles[k], start=(k == 0),
                                  stop=(k == FT - 1))
        return ps

    def proj_tm_psum(self, w_tiles, xn_tiles, t, bias_row):
        ps = self.p_mm.tile([128, CHUNK], F32, tag="mm", name="mm")
        self.nc.tensor.matmul(ps, self.ONESC, bias_row, start=True, stop=False)
        for k in range(FT):
            self.nc.tensor.matmul(ps, xn_tiles[k][:, 128 * t : 128 * (t + 1)],
                                  w_tiles[k], start=False, stop=(k == FT - 1))
        return ps

    def softmax_token_major(self, kps, kvp, ketmp, smallp):
        nc = self.nc
        kE = ketmp.tile([128, C], F32, tag="kE", name="kE")
        nc.scalar.activation(out=kE, in_=kps, func=AF.Exp)
        ssum = smallp.tile([128, H], F32, tag="ssum", name="ssum")
        nc.vector.tensor_reduce(
            out=ssum, in_=kE.rearrange("p (h d) -> p h d", d=D),
            axis=mybir.AxisListType.X, op=OP.add)
        rsum = smallp.tile([128, H], F32, tag="rsum", name="rsum")
        nc.vector.reciprocal(out=rsum, in_=ssum)
        kn = kvp.tile([128, C], F32R, tag="kn", name="kn")
        with nc.allow_low_precision(reason="fp32r feeds matmul"):
            for h in range(H):
                nc.vector.tensor_scalar(
                    out=kn[:, D * h : D * (h + 1)],
                    in0=kE[:, D * h : D * (h + 1)],
                    scalar1=rsum[:, h : h + 1], scalar2=None, op0=OP.mult)
        return kn

    def load_w512(self, ap, pool, tag):
        tiles = []
        for k in range(FT):
            t = pool.tile([128, C], F32R, tag=f"{tag}{k}", name=f"{tag}{k}")
            self.nc.sync.dma_start(
                out=t, in_=ap[128 * k : 128 * (k + 1), :].bitcast(F32R))
            tiles.append(t)
        return tiles

    def attn_front(self, Xin, wq_ap, bq_cols, lng, lnb):
        """LN + q-projection + exp for all chunks -> E tiles."""
        nc = self.nc
        E = [[None] * FT for _ in range(NCH)]
        with self.tc.tile_pool(name="w_q", bufs=1) as w_q:
            WQ = self.load_w512(wq_ap, w_q, "wq")
            for ch in range(NCH):
                xn = self.layer_norm(Xin[ch], lng, lnb)
                for m in range(FT):
                    ps = self.proj_fm_psum(WQ, xn, m)
                    e = self.epool.tile([128, CHUNK], F32R, tag="E", name="E")
                    nc.scalar.activation(out=e, in_=ps, func=AF.Exp,
                                         bias=bq_cols[:, m : m + 1], scale=1.0)
                    E[ch][m] = e
        return E

    def attn_back(self, Xin, E, cc_out, n_in, wo_ap, bo_cols, sg_w, new_resid):
        """SG/G reciprocals, broadcasts, block-diag apply, assembly, wo
        projection + residual. cc_out: DRAM tile ([n_in,65,C] or [65,C])."""
        nc, tc, I = self.nc, self.tc, self.I
        Xout = [[None] * FT for _ in range(NCH)]
        cc = (lambda i: cc_out[i]) if n_in > 1 else (lambda i: cc_out)
        with tc.tile_pool(name=f"w_{sg_w}", bufs=1) as w_o, \
             tc.tile_pool(name=f"as_{sg_w}", bufs=1) as attn_s, \
             tc.tile_pool(name=f"tmp_{sg_w}", bufs=4) as atmp, \
             tc.tile_pool(name=f"rec_{sg_w}", bufs=3) as recp:
            WO = self.load_w512(wo_ap, w_o, "wo")
            ncols = 8 + 8 * n_in
            SGT = []
            for c in range(FT):
                sg = attn_s.tile([128, ncols], F32R, tag=f"sgt{c}", name=f"sgt{c}")
                nc.sync.dma_start(
                    out=sg, in_=I["sgbase"][c][:, 0:ncols].bitcast(F32R))
                for i in range(n_in):
                    col = 8 + 8 * i + 2 * c
                    nc.gpsimd.dma_start(
                        out=sg[0:D, col : col + 1],
                        in_=cc(i)[D, 128 * c : 128 * c + D].rearrange(
                            "(p o) -> p o", o=1).bitcast(F32R))
                    nc.gpsimd.dma_start(
                        out=sg[D:128, col + 1 : col + 2],
                        in_=cc(i)[D, 128 * c + D : 128 * (c + 1)].rearrange(
                            "(p o) -> p o", o=1).bitcast(F32R))
                SGT.append(sg)
            BD = [[None] * FT for _ in range(n_in)]
            for i in range(n_in):
                for c in range(FT):
                    bd = attn_s.tile([128, 128], F32R, tag=f"bd{i}_{c}", name=f"bd{i}_{c}")
                    nc.sync.dma_start(out=bd, in_=I["zz"].bitcast(F32R))
                    nc.gpsimd.dma_start(
                        out=bd[0:D, 0:D],
                        in_=cc(i)[0:D, (2 * c) * D : (2 * c + 1) * D].bitcast(F32R))
                    nc.gpsimd.dma_start(
                        out=bd[D:128, D:128],
                        in_=cc(i)[0:D, (2 * c + 1) * D : (2 * c + 2) * D].bitcast(F32R))
                    BD[i][c] = bd

            for ch in range(NCH):
                recs = []
                for j in range(1 + n_in):
                    gps = self.p_stats.tile([8, CHUNK], F32, tag="stats", name="stats")
                    for c in range(FT):
                        nc.tensor.matmul(gps, SGT[c][:, 8 * j : 8 * (j + 1)],
                                         E[ch][c], start=(c == 0),
                                         stop=(c == FT - 1))
                    r = recp.tile([8, CHUNK], F32, tag="rec", name="rec")
                    nc.vector.reciprocal(out=r, in_=gps)
                    rr = recp.tile([8, CHUNK], F32R, tag="recr", name="recr")
                    nc.scalar.activation(out=rr, in_=r, func=AF.Copy, bias=0.0,
                                         scale=1.0)
                    recs.append(rr)
                outc = []
                for c in range(FT):
                    aps = []
                    gsb = []
                    for i in range(n_in):
                        a = self.p_mm.tile([128, CHUNK], F32, tag="mm", name="mm")
                        nc.tensor.matmul(a, BD[i][c], E[ch][c], start=True,
                                         stop=True)
                        asb_ = atmp.tile([128, CHUNK], F32, tag="apb", name="apb")
                        nc.scalar.activation(out=asb_, in_=a, func=AF.Copy,
                                             bias=0.0, scale=1.0)
                        aps.append(asb_)
                        gb = self.p_bc.tile([128, CHUNK], F32, tag="bc", name="bc")
                        nc.tensor.matmul(gb, self.SEL8[c], recs[1 + i],
                                         start=True, stop=True)
                        gs = atmp.tile([128, CHUNK], F32, tag="gbs", name="gbs")
                        nc.scalar.activation(out=gs, in_=gb, func=AF.Copy,
                                             bias=0.0, scale=1.0)
                        gsb.append(gs)
                    sb = self.p_bc.tile([128, CHUNK], F32, tag="bc", name="bc")
                    nc.tensor.matmul(sb, self.SEL8[c], recs[0], start=True,
                                     stop=True)
                    ssb = atmp.tile([128, CHUNK], F32, tag="gbs", name="gbs")
                    nc.scalar.activation(out=ssb, in_=sb, func=AF.Copy,
                                         bias=0.0, scale=1.0)
                    acc = atmp.tile([128, CHUNK], F32, tag="asm", name="asm")
                    nc.vector.tensor_tensor(out=acc, in0=E[ch][c].bitcast(F32),
                                            in1=ssb, op=OP.mult)
                    for i in range(n_in):
                        ai = atmp.tile([128, CHUNK], F32, tag="asm", name="asm")
                        nc.vector.tensor_tensor(out=ai, in0=gsb[i], in1=aps[i],
                                                op=OP.mult)
                        last = (i == n_in - 1)
                        nxt = self.xnp.tile([128, CHUNK], F32R, tag="xn", name="xn") if last \
                            else atmp.tile([128, CHUNK], F32, tag="asm", name="asm")
                        with nc.allow_low_precision(reason="fp32r feeds matmul"):
                            nc.vector.tensor_tensor(
                                out=nxt, in0=acc.bitcast(F32), in1=ai, op=OP.add)
                        acc = nxt
                    outc.append(acc)
                for m in range(FT):
                    wps = self.proj_fm_psum(WO, outc, m)
                    tt = self.wotp.tile([128, CHUNK], F32, tag="wot", name="wot")
                    nc.scalar.activation(out=tt, in_=wps, func=AF.Identity,
                                         bias=bo_cols[:, m : m + 1], scale=1.0)
                    xo = new_resid()
                    with nc.allow_low_precision(reason="fp32r feeds matmul"):
                        nc.vector.tensor_tensor(out=xo,
                                                in0=Xin[ch][m].bitcast(F32),
                                                in1=tt, op=OP.add)
                    Xout[ch][m] = xo
        return Xout

    def ffn(self, Xin, w1name, w2name, B1, B2, lng, lnb):
        nc, tc, I = self.nc, self.tc, self.I
        Xout = [[None] * FT for _ in range(NCH)]
        with tc.tile_pool(name=w1name, bufs=1) as w1p, \
             tc.tile_pool(name=w2name + "s", bufs=6) as w2p, \
             tc.tile_pool(name=w1name + "h", bufs=4) as hp, \
             tc.tile_pool(name=w1name + "p", bufs=4, space="PSUM") as p_ffn:
            W1 = []
            for k in range(FT):
                t = w1p.tile([128, INNER], F32R, tag=f"w1_{k}", name=f"w1_{k}")
                nc.sync.dma_start(
                    out=t, in_=I[w1name][128 * k : 128 * (k + 1), :].bitcast(F32R))
                W1.append(t)
            for ch in range(NCH):
                xn = self.layer_norm(Xin[ch], lng, lnb)
                ops = [p_ffn.tile([128, CHUNK], F32, tag="ffn", name="ffn")
                       for _ in range(FT)]
                for k in range(IT):
                    hps = self.p_mm.tile([128, CHUNK], F32, tag="mm", name="mm")
                    for c in range(FT):
                        nc.tensor.matmul(hps, W1[c][:, 128 * k : 128 * (k + 1)],
                                         xn[c], start=(c == 0),
                                         stop=(c == FT - 1))
                    h = hp.tile([128, CHUNK], F32R, tag="h", name="h")
                    nc.scalar.activation(out=h, in_=hps, func=AF.Gelu_apprx_tanh,
                                         bias=B1[:, k : k + 1], scale=1.0)
                    w2t = w2p.tile([128, C], F32R, tag="w2s", name="w2s")
                    nc.sync.dma_start(
                        out=w2t,
                        in_=I[w2name][128 * k : 128 * (k + 1), :].bitcast(F32R))
                    for m in range(FT):
                        nc.tensor.matmul(ops[m],
                                         w2t[:, 128 * m : 128 * (m + 1)], h,
                                         start=(k == 0), stop=(k == IT - 1))
                for m in range(FT):
                    tt = self.wotp.tile([128, CHUNK], F32, tag="wot", name="wot")
                    nc.scalar.activation(out=tt, in_=ops[m], func=AF.Identity,
                                         bias=B2[:, m : m + 1], scale=1.0)
                    xo = self.resid.tile([128, CHUNK], F32R, tag="resid", name="resid")
                    with nc.allow_low_precision(reason="fp32r feeds matmul"):
                        nc.vector.tensor_tensor(out=xo,
                                                in0=Xin[ch][m].bitcast(F32),
                                                in1=tt, op=OP.add)
                    Xout[ch][m] = xo
        return Xout

    # ---------------- main ----------------
    def run(self):
        nc, tc, I = self.nc, self.tc, self.I
        from contextlib import ExitStack

        with ExitStack() as ctx:
            const = ctx.enter_context(tc.tile_pool(name="const", bufs=1))
            self.resid = ctx.enter_context(tc.tile_pool(name="resid", bufs=20))
            self.epool = ctx.enter_context(tc.tile_pool(name="E", bufs=16))
            self.xnp = ctx.enter_context(tc.tile_pool(name="xn", bufs=5))
            self.rows = ctx.enter_context(tc.tile_pool(name="rows", bufs=8))
            self.lntmp = ctx.enter_context(tc.tile_pool(name="lntmp", bufs=3))
            self.wotp = ctx.enter_context(tc.tile_pool(name="wot", bufs=3))
            dram = ctx.enter_context(tc.tile_pool(name="dram", bufs=1,
                                                  space="DRAM"))
            self.p_mm = ctx.enter_context(
                tc.tile_pool(name="p_mm", bufs=2, space="PSUM"))
            self.p_stats = ctx.enter_context(
                tc.tile_pool(name="p_stats", bufs=1, space="PSUM"))
            self.p_bc = ctx.enter_context(
                tc.tile_pool(name="p_bc", bufs=1, space="PSUM"))

            # ---------------- constants ----------------
            self.EPS = const.tile([1, 1], F32, tag="eps", name="eps")
            nc.vector.memset(self.EPS, LN_EPS)
            self.ONESC = const.tile([1, 128], F32R, tag="onesc", name="onesc")
            nc.sync.dma_start(out=self.ONESC, in_=I["ones_c"].bitcast(F32R))
            self.ONESR = const.tile([128, 1], F32R, tag="onesr", name="onesr")
            nc.sync.dma_start(out=self.ONESR, in_=I["ones_r"].bitcast(F32R))
            self.SEL8 = []
            for c in range(FT):
                s = const.tile([8, 128], F32R, tag=f"sel8_{c}", name=f"sel8_{c}")
                nc.sync.dma_start(out=s, in_=I["sel8"][c].bitcast(F32R))
                self.SEL8.append(s)

            def cols_tile(name, nt):
                t = const.tile([128, nt], F32, tag=name)
                nc.sync.dma_start(out=t, in_=I[name])
                return t

            BQ = cols_tile("bq_c", FT)
            BO = cols_tile("bo_c", FT)
            SAQ = cols_tile("saq_c", FT)
            SAO = cols_tile("sao_c", FT)
            F1B1 = cols_tile("f1b1_c", IT)
            F1B2 = cols_tile("f1b2_c", FT)
            F2B1 = cols_tile("f2b1_c", IT)
            F2B2 = cols_tile("f2b2_c", FT)
            LNG, LNB = {}, {}
            for lname in ["ln1", "ln3", "ln4", "ln5"]:
                LNG[lname] = cols_tile(lname + "_g", FT)
                LNB[lname] = cols_tile(lname + "_b", FT)
            for i in range(NIN):
                g = const.tile([128, FT], F32, tag=f"ln2g{i}", name=f"ln2g{i}")
                nc.sync.dma_start(out=g, in_=I["ln2_g"][i])
                b = const.tile([128, FT], F32, tag=f"ln2b{i}", name=f"ln2b{i}")
                nc.sync.dma_start(out=b, in_=I["ln2_b"][i])
                LNG[f"ln2_{i}"], LNB[f"ln2_{i}"] = g, b

            def row_tile(apslice, tag):
                t = const.tile([1, C], F32R, tag=tag)
                nc.sync.dma_start(out=t, in_=apslice.bitcast(F32R))
                return t

            BKR = [row_tile(I["bk_r"][i], f"bkr{i}") for i in range(NIN)]
            BVR = [row_tile(I["bv_r"][i], f"bvr{i}") for i in range(NIN)]
            SAKR = row_tile(I["sak_r"], "sakr")
            SAVR = row_tile(I["sav_r"], "savr")

            # ---------------- residual load ----------------
            X = [[self.resid.tile([128, CHUNK], F32R, tag="resid", name="resid")
                  for _ in range(FT)] for _ in range(NCH)]
            for ch in range(NCH):
                for c in range(FT):
                    nc.sync.dma_start(
                        out=X[ch][c],
                        in_=I["xT"][128 * c : 128 * (c + 1),
                                    CHUNK * ch : CHUNK * (ch + 1)].bitcast(F32R))

            # ============ phase A: CA front ============
            E = self.attn_front(X, I["wq"], BQ, LNG["ln1"], LNB["ln1"])
            cc_in = dram.tile([NIN, D + 1, C], F32, tag="cc_ca_in", name="cc_ca_in")
            cc_out = dram.tile([NIN, D + 1, C], F32, tag="cc_ca_out", name="cc_ca_out")
            with tc.tile_pool(name="w_kv", bufs=1) as w_kv, \
                 tc.tile_pool(name="ysp", bufs=4) as ysp, \
                 tc.tile_pool(name="kvp", bufs=2) as kvp, \
                 tc.tile_pool(name="kep", bufs=2) as kep, \
                 tc.tile_pool(name="smallp", bufs=4) as smallp, \
                 tc.tile_pool(name="ctxsb", bufs=1) as ctxsbp, \
                 tc.tile_pool(name="p_ctx", bufs=2, space="PSUM") as p_ctx, \
                 tc.tile_pool(name="p_ks", bufs=2, space="PSUM") as p_ks:
                WK = [self.load_w512(I["wk"][i], w_kv, f"wk{i}")
                      for i in range(NIN)]
                WV = [self.load_w512(I["wv"][i], w_kv, f"wv{i}")
                      for i in range(NIN)]
                CTXA = [ctxsbp.tile([D, C], F32, tag=f"ctxacc{i}",
                                    name=f"ctxacc{i}") for i in range(NIN)]
                KSA = [ctxsbp.tile([1, C], F32, tag=f"ksacc{i}",
                                   name=f"ksacc{i}") for i in range(NIN)]
                for ch in range(NCH):
                    for i in range(NIN):
                        yt = []
                        for c in range(FT):
                            y = ysp.tile([128, CHUNK], F32R, tag="ys", name="ys")
                            nc.sync.dma_start(
                                out=y,
                                in_=I["ysT"][i, 128 * c : 128 * (c + 1),
                                             CHUNK * ch : CHUNK * (ch + 1)
                                             ].bitcast(F32R))
                            yt.append(y)
                        yn = self.layer_norm(yt, LNG[f"ln2_{i}"],
                                             LNB[f"ln2_{i}"])
                        ctx_ps = p_ctx.tile([D, C], F32, tag="ctx", name="ctx")
                        ks_ps = p_ks.tile([1, C], F32, tag="ks", name="ks")
                        for t in range(FT):
                            kps = self.proj_tm_psum(WK[i], yn, t, BKR[i])
                            kn = self.softmax_token_major(kps, kvp, kep, smallp)
                            vps = self.proj_tm_psum(WV[i], yn, t, BVR[i])
                            vn = kvp.tile([128, C], F32R, tag="vn", name="vn")
                            nc.scalar.activation(out=vn, in_=vps, func=AF.Copy,
                                                 bias=0.0, scale=1.0)
                            for h in range(H):
                                nc.tensor.matmul(
                                    ctx_ps[:, D * h : D * (h + 1)],
                                    kn[:, D * h : D * (h + 1)],
                                    vn[:, D * h : D * (h + 1)],
                                    start=(t == 0 and h == 0),
                                    stop=(t == FT - 1 and h == H - 1))
                            nc.tensor.matmul(ks_ps, self.ONESR, kn,
                                             start=(t == 0),
                                             stop=(t == FT - 1))
                        if ch == 0:
                            nc.vector.tensor_copy(CTXA[i], ctx_ps)
                            nc.vector.tensor_copy(KSA[i], ks_ps)
                        else:
                            nc.vector.tensor_tensor(out=CTXA[i], in0=CTXA[i],
                                                    in1=ctx_ps, op=OP.add)
                            nc.vector.tensor_tensor(out=KSA[i], in0=KSA[i],
                                                    in1=ks_ps, op=OP.add)
                for i in range(NIN):
                    nc.sync.dma_start(out=cc_in[i, 0:D, :], in_=CTXA[i])
                    nc.sync.dma_start(out=cc_in[i, D : D + 1, :], in_=KSA[i])
            nc.gpsimd.collective_compute(
                "AllReduce", OP.add, replica_groups=GROUPS,
                ins=[cc_in[:].opt()], outs=[cc_out[:].opt()])

            # ============ phase B: CA back + FFN1 ============
            X1 = self.attn_back(
                X, E, cc_out, NIN, I["wo"], BO, "ca",
                lambda: self.resid.tile([128, CHUNK], F32R, tag="resid", name="resid"))
            X2 = self.ffn(X1, "f1w1", "f1w2", F1B1, F1B2, LNG["ln3"],
                          LNB["ln3"])

            # ============ phase C: SA front ============
            E2 = self.attn_front(X2, I["saq"], SAQ, LNG["ln4"], LNB["ln4"])
            cc2_in = dram.tile([D + 1, C], F32, tag="cc_sa_in", name="cc_sa_in")
            cc2_out = dram.tile([D + 1, C], F32, tag="cc_sa_out", name="cc_sa_out")
            with tc.tile_pool(name="w_kv2", bufs=1) as w_kv2, \
                 tc.tile_pool(name="kvp2", bufs=2) as kvp2, \
                 tc.tile_pool(name="kep2", bufs=2) as kep2, \
                 tc.tile_pool(name="smallp2", bufs=4) as smallp2, \
                 tc.tile_pool(name="ctxsb2", bufs=1) as ctxsbp2, \
                 tc.tile_pool(name="p_ctx2", bufs=1, space="PSUM") as p_ctx2, \
                 tc.tile_pool(name="p_ks2", bufs=1, space="PSUM") as p_ks2:
                SWK = self.load_w512(I["sak"], w_kv2, "sak")
                SWV = self.load_w512(I["sav"], w_kv2, "sav")
                CTXA2 = ctxsbp2.tile([D, C], F32, tag="ctxacc2", name="ctxacc2")
                KSA2 = ctxsbp2.tile([1, C], F32, tag="ksacc2", name="ksacc2")
                for ch in range(NCH):
                    xn = self.layer_norm(X2[ch], LNG["ln4"], LNB["ln4"])
                    ctx_ps = p_ctx2.tile([D, C], F32, tag="ctx2", name="ctx2")
                    ks_ps = p_ks2.tile([1, C], F32, tag="ks2", name="ks2")
                    for t in range(FT):
                        kps = self.proj_tm_psum(SWK, xn, t, SAKR)
                        kn = self.softmax_token_major(kps, kvp2, kep2, smallp2)
                        vps = self.proj_tm_psum(SWV, xn, t, SAVR)
                        vn = kvp2.tile([128, C], F32R, tag="vn", name="vn")
                        nc.scalar.activation(out=vn, in_=vps, func=AF.Copy,
                                             bias=0.0, scale=1.0)
                        for h in range(H):
                            nc.tensor.matmul(
                                ctx_ps[:, D * h : D * (h + 1)],
                                kn[:, D * h : D * (h + 1)],
                                vn[:, D * h : D * (h + 1)],
                                start=(t == 0 and h == 0),
                                stop=(t == FT - 1 and h == H - 1))
                        nc.tensor.matmul(ks_ps, self.ONESR, kn,
                                         start=(t == 0),
                                         stop=(t == FT - 1))
                    if ch == 0:
                        nc.vector.tensor_copy(CTXA2, ctx_ps)
                        nc.vector.tensor_copy(KSA2, ks_ps)
                    else:
                        nc.vector.tensor_tensor(out=CTXA2, in0=CTXA2,
                                                in1=ctx_ps, op=OP.add)
                        nc.vector.tensor_tensor(out=KSA2, in0=KSA2,
                                                in1=ks_ps, op=OP.add)
                nc.sync.dma_start(out=cc2_in[0:D, :], in_=CTXA2)
                nc.sync.dma_start(out=cc2_in[D : D + 1, :], in_=KSA2)
            nc.gpsimd.collective_compute(
                "AllReduce", OP.add, replica_groups=GROUPS,
                ins=[cc2_in[:].opt()], outs=[cc2_out[:].opt()])

            # ============ phase D: SA back + FFN2 ============
            X3 = self.attn_back(
                X2, E2, cc2_out, 1, I["sao"], SAO, "sa",
                lambda: self.resid.tile([128, CHUNK], F32R, tag="resid", name="resid"))
            XF = self.ffn(X3, "f2w1", "f2w2", F2B1, F2B2, LNG["ln5"],
                          LNB["ln5"])

            for ch in range(NCH):
                for m in range(FT):
                    nc.sync.dma_start(
                        out=self.out_t[128 * m : 128 * (m + 1),
                                       CHUNK * ch : CHUNK * (ch + 1)],
                        in_=XF[ch][m].bitcast(F32))


# ---------------------------------------------------------------------------
# host side
# ---------------------------------------------------------------------------
_PROGRAM = None
LAST_RESULTS = None


def _cols(v, nt):
    return np.ascontiguousarray(np.asarray(v, np.float32).reshape(nt, 128).T)


def _host_consts():
    sgbase = np.zeros((FT, 128, 24), np.float32)
    sel8 = np.zeros((FT, 8, 128), np.float32)
    for c in range(FT):
        for p in range(128):
            h = 2 * c + (1 if p >= 64 else 0)
            sgbase[c, p, h] = 1.0
            sel8[c, h, p] = 1.0
    return {
        "ones_c": np.ones((1, 128), np.float32),
        "ones_r": np.ones((128, 1), np.float32),
        "sgbase": sgbase,
        "sel8": sel8,
        "zz": np.zeros((128, 128), np.float32),
    }


def _make_in_maps(inputs):
    f = lambda k: np.asarray(inputs[k], np.float32)
    shared = {
        "wq": np.ascontiguousarray(f("ca_wq").T),
        "wo": np.ascontiguousarray(f("ca_wo").T),
        "saq": np.ascontiguousarray(f("sa_wq").T),
        "sak": np.ascontiguousarray(f("sa_wk").T),
        "sav": np.ascontiguousarray(f("sa_wv").T),
        "sao": np.ascontiguousarray(f("sa_wo").T),
        "wk": np.ascontiguousarray(f("ca_wk").transpose(0, 2, 1)),
        "wv": np.ascontiguousarray(f("ca_wv").transpose(0, 2, 1)),
        "f1w1": np.ascontiguousarray(f("ffn1_w1").T),
        "f1w2": np.ascontiguousarray(f("ffn1_w2").T),
        "f2w1": np.ascontiguousarray(f("ffn2_w1").T),
        "f2w2": np.ascontiguousarray(f("ffn2_w2").T),
        "bq_c": _cols(f("ca_bq"), FT),
        "bo_c": _cols(f("ca_bo"), FT),
        "saq_c": _cols(f("sa_bq"), FT),
        "sao_c": _cols(f("sa_bo"), FT),
        "f1b1_c": _cols(f("ffn1_b1"), IT),
        "f1b2_c": _cols(f("ffn1_b2"), FT),
        "f2b1_c": _cols(f("ffn2_b1"), IT),
        "f2b2_c": _cols(f("ffn2_b2"), FT),
        "bk_r": np.ascontiguousarray(f("ca_bk").reshape(NIN, 1, C)),
        "bv_r": np.ascontiguousarray(f("ca_bv").reshape(NIN, 1, C)),
        "sak_r": np.ascontiguousarray(f("sa_bk").reshape(1, C)),
        "sav_r": np.ascontiguousarray(f("sa_bv").reshape(1, C)),
        "ln1_g": _cols(f("ln1_g"), FT), "ln1_b": _cols(f("ln1_b"), FT),
        "ln3_g": _cols(f("ln3_g"), FT), "ln3_b": _cols(f("ln3_b"), FT),
        "ln4_g": _cols(f("ln4_g"), FT), "ln4_b": _cols(f("ln4_b"), FT),
        "ln5_g": _cols(f("ln5_g"), FT), "ln5_b": _cols(f("ln5_b"), FT),
        "ln2_g": np.stack([_cols(f("ln2_g")[i], FT) for i in range(NIN)]),
        "ln2_b": np.stack([_cols(f("ln2_b")[i], FT) for i in range(NIN)]),
    }
    shared.update(_host_consts())

    x = f("x")
    ys = f("ys")
    in_maps = []
    for core in range(N_CORES):
        b, half = core // 2, core % 2
        lo, hi = half * NTOK, (half + 1) * NTOK
        m = dict(shared)
        m["xT"] = np.ascontiguousarray(x[b, lo:hi, :].T)
        m["ysT"] = np.ascontiguousarray(ys[:, b, lo:hi, :].transpose(0, 2, 1))
        in_maps.append(m)
    return in_maps


def kernel(**inputs):
    global _PROGRAM, LAST_RESULTS
    if _PROGRAM is None:
        _PROGRAM = _build_program()
    nc = _PROGRAM
    in_maps = _make_in_maps(inputs)

    trace = os.environ.get("BASS_TRACE", "") not in ("", "0")
    res = run_bass_kernel_spmd(nc, in_maps, core_ids=list(range(N_CORES)),
                               trace=trace)
    LAST_RESULTS = res

    out = np.empty((B, T, C), np.float32)
    for core in range(N_CORES):
        b, half = core // 2, core % 2
        lo, hi = half * NTOK, (half + 1) * NTOK
        out[b, lo:hi, :] = res.results[core]["outT"].T
    return out



# revision 9
# speedup vs baseline: 1.2495x; 1.2495x over previous
"""Trainium2 Bass kernel for nn_CrossAttentionBlock (B=4, T=4096, C=512, H=8,
INNER=2048, NIN=2) on 8 NeuronCores.

Sharding: core c handles batch b=c//2, token half h=c%2 (2048 tokens each).
All per-token math is local; the only cross-core coupling is the linear-
attention context (ctx = k^T v, [H,64,64] per batch) and k_sum, reduced with
pair-wise AllReduces (cores 2b and 2b+1).

v2 design (all-bf16 matmul datapath):
- Every matmul runs in bf16 (weights pre-cast host-side, activations written
  as bf16 by the producing Act/DVE/Pool op). PSUM accumulation stays fp32.
  This avoids the throttled fp32_mode=HIGH PE path and halves LDWEIGHTS.
- LayerNorm affine (g, b) is folded into the consuming projection weights on
  the host (rows scaled by g, b folded into the output bias), so on-chip LN
  only computes per-token a=rsqrt(var+eps) and mean, then xs = x*aB - amB.
- Per-token LN stats use fast fp32 reciprocal_approx_fast (never the 3.3us
  fp32r-output reciprocal path); rsqrt = recip(sqrt()).
- Loops are batched by activation function (Sqrt / Exp / Gelu) to minimize
  1.28us ACT_TABLE_LOADs, and chunks pipeline across engines.
- Elementwise work is spread across DVE and the otherwise-idle Pool engine.
- k/v sides run before the q sides so the q projections + exp overlap the
  AllReduce; SG/BD stationaries are built with compute-engine copies from a
  staged bf16 ctx tile instead of many small scatter DMAs.
- All weights live in SBUF for the whole kernel (no re-streaming).
"""
import os
import numpy as np

import concourse.bass as bass
import concourse.tile as tile
from concourse import mybir
from concourse.vector_clock import ScopedClock
from concourse.bass_utils import run_bass_kernel_spmd

F32 = mybir.dt.float32
BF16 = mybir.dt.bfloat16
AF = mybir.ActivationFunctionType
OP = mybir.AluOpType

B, T, C, H, D, INNER, NIN = 4, 4096, 512, 8, 64, 2048, 2
N_CORES = 8
NTOK = 2048          # tokens per core
CHUNK = 512          # tokens per chunk
NCH = NTOK // CHUNK  # 4 chunks
FT = C // 128        # 4 feature tiles
IT = INNER // 128    # 16 inner tiles
LN_EPS = 1e-5
GROUPS = [[0, 1], [2, 3], [4, 5], [6, 7]]

_split_counter = [0]


def _split_multi_waits(nc):
    """This walrus build only supports one sync-wait per instruction; move
    extra waits onto same-engine NoOps placed immediately before."""
    for f in nc.m.functions:
        for blk in f.blocks:
            out = []
            changed = False
            for inst in blk.instructions:
                si = inst.sync_info
                if si is not None and si.on_wait and len(si.on_wait) > 1:
                    waits = list(si.on_wait)
                    for w in waits[:-1]:
                        _split_counter[0] += 1
                        nop = mybir.InstNoOp(
                            name=f"I-waitsplit-{_split_counter[0]}", ins=[], outs=[]
                        )
                        nop.engine = inst.engine
                        nop.sync_info = mybir.SyncInfo(on_wait=[w], on_update=[])
                        out.append(nop)
                    si.on_wait = waits[-1:]
                    inst.sync_info = si
                    changed = True
                out.append(inst)
            if changed:
                blk.instructions = out


class _TC(tile.TileContext):
    def _drain_and_barrier(self, tick_clock, wait_clock):
        drain_inst = self.nc.sync.drain()
        wait_clock.add_sem_waits(
            drain_inst.ins, ScopedClock({None: tick_clock.global_clock})
        )
        si = drain_inst.ins.sync_info
        if si is not None and si.on_wait and len(si.on_wait) > 1:
            waits = list(si.on_wait)
            si.on_wait = waits[:1]
            drain_inst.ins.sync_info = si
            for i in range(1, len(waits)):
                extra = self.nc.sync.drain()
                esi = extra.ins.sync_info
                if esi is None:
                    extra.ins.sync_info = mybir.SyncInfo(
                        on_wait=waits[i : i + 1], on_update=[]
                    )
                else:
                    esi.on_wait = waits[i : i + 1]
                    extra.ins.sync_info = esi
        self.nc.all_engine_barrier()
        assert self.sems is not None
        popped = self.nc._tile_sem_poison_stack.pop()
        assert popped is self._sem_poison
        self.nc.clear_and_free_semaphores(list(self.sems.allocated().values()))
        self.nc.all_engine_barrier()


def _build_program(split=True):
    nc = bass.Bass("TRN2", target_bir_lowering=False, debug=False, num_devices=N_CORES)
    I = {}

    def di(name, shape, dt=BF16):
        I[name] = nc.dram_tensor(name, list(shape), dt, kind="ExternalInput").ap()

    di("xT", [C, NTOK])
    di("ysT", [NIN, C, NTOK])
    for w in ["wq", "wo", "saq", "sak", "sav", "sao"]:
        di(w, [C, C])
    di("wk", [NIN, C, C])
    di("wv", [NIN, C, C])
    di("f1w1", [C, INNER])
    di("f1w2", [INNER, C])
    di("f2w1", [C, INNER])
    di("f2w2", [INNER, C])
    for bname in ["bq_c", "bo_c", "saq_c", "sao_c", "f1b2_c", "f2b2_c"]:
        di(bname, [128, FT], F32)
    di("f1b1_c", [128, IT], F32)
    di("f2b1_c", [128, IT], F32)
    di("bk_r", [NIN, 1, C])
    di("bv_r", [NIN, 1, C])
    di("sak_r", [1, C])
    di("sav_r", [1, C])
    di("ones_c", [1, 128])
    di("ones_r", [128, 1])
    di("sgbase", [FT, 128, 72])
    di("sel24", [FT, 72, 128])

    out_t = nc.dram_tensor("outT", [C, NTOK], F32, kind="ExternalOutput").ap()

    with _TC(nc) as tc:
        with nc.allow_low_precision(reason="bf16 datapath, tolerance 2e-2"):
            _Emitter(nc, tc, I, out_t).run()
    if split:
        _split_multi_waits(nc)
    return nc


class _Emitter:
    def __init__(self, nc, tc, I, out_t):
        self.nc, self.tc, self.I, self.out_t = nc, tc, I, out_t

    # ---------------- layer norm (folded affine) ----------------
    def ln_batch(self, Xin, chunks=None):
        """LN stats + xs = (x - m) * a for each chunk. Returns xs tiles
        (bf16) per chunk. The g/b affine is folded into downstream weights
        host-side. Act func used: Sqrt only."""
        nc = self.nc
        if chunks is None:
            chunks = range(NCH)
        XS = {}
        for ch in chunks:
            x = Xin[ch]
            s_ps = self.p_stats.tile([1, CHUNK], F32, tag="stats", name="stats")
            for k in range(FT):
                nc.tensor.matmul(s_ps, self.ONESR, x[k],
                                 start=(k == 0), stop=(k == FT - 1))
            xsq = []
            for k in range(FT):
                sq = self.lntmp.tile([128, CHUNK], BF16, tag="xsq", name="xsq")
                nc.gpsimd.tensor_tensor(out=sq, in0=x[k], in1=x[k], op=OP.mult)
                xsq.append(sq)
            q_ps = self.p_stats.tile([1, CHUNK], F32, tag="stats", name="stats")
            for k in range(FT):
                nc.tensor.matmul(q_ps, self.ONESR, xsq[k],
                                 start=(k == 0), stop=(k == FT - 1))
            mrow = self.rows.tile([1, CHUNK], F32, tag="rows", name="rows")
            nc.vector.tensor_scalar(out=mrow, in0=s_ps, scalar1=1.0 / C,
                                    scalar2=None, op0=OP.mult)
            s2 = self.rows.tile([1, CHUNK], F32, tag="rows", name="rows")
            nc.gpsimd.tensor_tensor(out=s2, in0=mrow, in1=mrow, op=OP.mult)
            v = self.rows.tile([1, CHUNK], F32, tag="rows", name="rows")
            nc.vector.scalar_tensor_tensor(out=v, in0=q_ps, scalar=1.0 / C,
                                           in1=s2, op0=OP.mult,
                                           op1=OP.subtract)
            sq_ = self.rows.tile([1, CHUNK], F32, tag="rows", name="rows")
            nc.scalar.activation(out=sq_, in_=v, func=AF.Sqrt, bias=self.EPS,
                                 scale=1.0)
            a = self.rows.tile([1, CHUNK], F32, tag="rows", name="rows")
            nc.vector.reciprocal(out=a, in_=sq_)
            am = self.rowsb.tile([1, CHUNK], BF16, tag="rowsb", name="rowsb")
            nc.gpsimd.tensor_tensor(out=am, in0=a, in1=mrow, op=OP.mult)
            a_bf = self.rowsb.tile([1, CHUNK], BF16, tag="rowsb", name="rowsb")
            nc.gpsimd.tensor_copy(a_bf, a)
            bc_ps = self.p_bc.tile([128, CHUNK], F32, tag="bc", name="bc")
            nc.tensor.matmul(bc_ps, self.ONESC, a_bf, start=True, stop=True)
            aB = self.lnab.tile([128, CHUNK], BF16, tag="aB", name="aB")
            nc.scalar.activation(out=aB, in_=bc_ps, func=AF.Copy, bias=0.0,
                                 scale=1.0)
            bc_ps2 = self.p_bc.tile([128, CHUNK], F32, tag="bc", name="bc")
            nc.tensor.matmul(bc_ps2, self.ONESC, am, start=True, stop=True)
            amB = self.lnab.tile([128, CHUNK], BF16, tag="amB", name="amB")
            nc.vector.tensor_copy(amB, bc_ps2)
            xs = []
            for k in range(FT):
                t1 = self.lntmp.tile([128, CHUNK], BF16, tag="lnt", name="lnt")
                nc.vector.tensor_tensor(out=t1, in0=x[k], in1=aB, op=OP.mult)
                xk = self.xnp.tile([128, CHUNK], BF16, tag="xn", name="xn")
                nc.gpsimd.tensor_tensor(out=xk, in0=t1, in1=amB,
                                        op=OP.subtract)
                xs.append(xk)
            XS[ch] = xs
        return XS

    # ---------------- matmul helpers ----------------
    def proj_fm_psum(self, w_tiles, xs, m):
        ps = self.p_mm.tile([128, CHUNK], F32, tag="mm", name="mm")
        for k in range(FT):
            self.nc.tensor.matmul(ps, w_tiles[k][:, 128 * m : 128 * (m + 1)],
                                  xs[k], start=(k == 0), stop=(k == FT - 1))
        return ps

    def proj_tm_psum(self, w_tiles, xs, t, bias_row):
        ps = self.p_mm.tile([128, CHUNK], F32, tag="mm", name="mm")
        self.nc.tensor.matmul(ps, self.ONESC, bias_row, start=True, stop=False)
        for k in range(FT):
            self.nc.tensor.matmul(ps, xs[k][:, 128 * t : 128 * (t + 1)],
                                  w_tiles[k], start=False, stop=(k == FT - 1))
        return ps

    def load_w512(self, ap, pool, tag):
        tiles = []
        for k in range(FT):
            t = pool.tile([128, C], BF16, tag=f"{tag}{k}", name=f"{tag}{k}")
            self.nc.sync.dma_start(out=t, in_=ap[128 * k : 128 * (k + 1), :])
            tiles.append(t)
        return tiles

    # ---------------- attention pieces ----------------
    def kv_side(self, xs_by_ch, WK, WV, bk_row, bv_row, ctx_ps, ks_acc):
        """k/v projections (token-major), k-softmax over d, ctx/ks
        accumulation. ctx accumulates in psum across all chunks; ks
        accumulates into SBUF (Pool adds). Act func: Exp + Copy."""
        nc = self.nc
        first = [True]
        for ch in range(NCH):
            xs = xs_by_ch[ch]
            ks_ps = self.p_stats.tile([1, C], F32, tag="stats", name="stats")
            for t in range(FT):
                kps = self.proj_tm_psum(WK, xs, t, bk_row)
                kE = self.kep.tile([128, C], BF16, tag="kE", name="kE")
                nc.scalar.activation(out=kE, in_=kps, func=AF.Exp)
                ssum = self.smallp.tile([128, H], F32, tag="ssum", name="ssum")
                nc.vector.tensor_reduce(
                    out=ssum, in_=kE.rearrange("p (h d) -> p h d", d=D),
                    axis=mybir.AxisListType.X, op=OP.add)
                rsum = self.smallp.tile([128, H], F32, tag="rsum", name="rsum")
                nc.vector.reciprocal(out=rsum, in_=ssum)
                kn = self.kvp.tile([128, C], BF16, tag="kn", name="kn")
                for h in range(H):
                    nc.vector.tensor_scalar(
                        out=kn[:, D * h : D * (h + 1)],
                        in0=kE[:, D * h : D * (h + 1)],
                        scalar1=rsum[:, h : h + 1], scalar2=None, op0=OP.mult)
                vps = self.proj_tm_psum(WV, xs, t, bv_row)
                vn = self.kvp.tile([128, C], BF16, tag="vn", name="vn")
                nc.scalar.activation(out=vn, in_=vps, func=AF.Copy, bias=0.0,
                                     scale=1.0)
                for h in range(H):
                    nc.tensor.matmul(
                        ctx_ps[:, D * h : D * (h + 1)],
                        kn[:, D * h : D * (h + 1)],
                        vn[:, D * h : D * (h + 1)],
                        start=(first[0] and h == 0),
                        stop=(ch == NCH - 1 and t == FT - 1 and h == H - 1))
                nc.tensor.matmul(ks_ps, self.ONESR, kn, start=(t == 0),
                                 stop=(t == FT - 1))
                first[0] = False
            if ch == 0:
                nc.vector.tensor_copy(ks_acc, ks_ps)
            else:
                nc.vector.tensor_tensor(out=ks_acc, in0=ks_acc, in1=ks_ps,
                                        op=OP.add)

    def front_E(self, xs_by_ch, W, b_cols, epool):
        """q projection + exp -> E tiles (bf16). Act func: Exp."""
        nc = self.nc
        E = [[None] * FT for _ in range(NCH)]
        for ch in range(NCH):
            for m in range(FT):
                ps = self.proj_fm_psum(W, xs_by_ch[ch], m)
                e = epool.tile([128, CHUNK], BF16, tag="E", name="E")
                nc.scalar.activation(out=e, in_=ps, func=AF.Exp,
                                     bias=b_cols[:, m : m + 1], scale=1.0)
                E[ch][m] = e
        return E

    def build_staged(self, cc_ap, n_in, staging):
        """From AllReduced cc ([n,65,C] or [65,C] fp32 DRAM): per input a
        [128,C] bf16 tile with ctx duplicated on both partition halves, a
        transposed ksum [128,FT] fp32 tile, and sg stationaries [128,ncols]
        bf16 per feature tile."""
        nc, I = self.nc, self.I
        cc = (lambda i: cc_ap[i]) if n_in > 1 else (lambda i: cc_ap)
        ncols = 32 * n_in + 8
        staged, sgt = [], []
        for c in range(FT):
            sg = staging.tile([128, ncols], BF16, tag=f"sg{c}", name=f"sg{c}")
            nc.sync.dma_start(out=sg, in_=I["sgbase"][c][:, 0:ncols])
            sgt.append(sg)
        for i in range(n_in):
            st = staging.tile([128, C], F32, tag=f"st{i}", name=f"st{i}")
            nc.sync.dma_start(out=st[0:D, :], in_=cc(i)[0:D, :])
            nc.sync.dma_start(out=st[D:128, :], in_=cc(i)[0:D, :])
            stb = staging.tile([128, C], BF16, tag=f"stb{i}", name=f"stb{i}")
            nc.scalar.activation(out=stb, in_=st, func=AF.Copy, bias=0.0,
                                 scale=1.0)
            kst = staging.tile([128, FT], F32, tag=f"kst{i}", name=f"kst{i}")
            nc.sync.dma_start(
                out=kst,
                in_=cc(i)[D : D + 1, :].rearrange("o (a p) -> p (o a)", p=128))
            for c in range(FT):
                col = 32 * (1 + i) + 2 * c
                nc.scalar.activation(
                    out=sgt[c][0:D, col : col + 1], in_=kst[0:D, c : c + 1],
                    func=AF.Copy, bias=0.0, scale=1.0)
                nc.scalar.activation(
                    out=sgt[c][D:128, col + 1 : col + 2],
                    in_=kst[D:128, c : c + 1],
                    func=AF.Copy, bias=0.0, scale=1.0)
            staged.append(stb)
        return staged, sgt

    def attn_back(self, Xin, E, staged, sgt, n_in, WO, bo_cols, out_pool):
        """Denominators (single M=8+8n SG matmul), reciprocal, broadcasts,
        per-head ctx apply (split-K from staged bf16), assembly, wo
        projection + residual."""
        nc, tc = self.nc, self.tc
        ncols = 32 * n_in + 8
        Xout = [[None] * FT for _ in range(NCH)]
        # --- all denominators + reciprocals (no act tables) ---
        RR = []
        p_sg_ctx = tc.tile_pool(name="p_sg", bufs=2, space="PSUM")
        p_sg = p_sg_ctx.__enter__()
        for ch in range(NCH):
            g_ps = p_sg.tile([ncols, CHUNK], F32, tag="sg", name="sg")
            for c in range(FT):
                nc.tensor.matmul(g_ps, sgt[c][:, 0:ncols], E[ch][c],
                                 start=(c == 0), stop=(c == FT - 1))
            rec = self.recp.tile([ncols, CHUNK], F32, tag="rec", name="rec")
            nc.vector.reciprocal(out=rec, in_=g_ps)
            rr = self.recb.tile([ncols, CHUNK], BF16, tag="recb", name="recb")
            nc.scalar.activation(out=rr, in_=rec, func=AF.Copy, bias=0.0,
                                 scale=1.0)
            RR.append(rr)
        p_sg_ctx.__exit__(None, None, None)
        # --- apply + assemble + project ---
        for ch in range(NCH):
            rr = RR[ch]
            outc = []
            for c in range(FT):
                sb_ps = self.p_bc.tile([128, CHUNK], F32, tag="bc", name="bc")
                nc.tensor.matmul(sb_ps, self.SEL24[c][0:8, :], rr[0:8, :],
                                 start=True, stop=True)
                ssb = self.atmp.tile([128, CHUNK], BF16, tag="gbs", name="gbs")
                nc.scalar.activation(out=ssb, in_=sb_ps, func=AF.Copy,
                                     bias=0.0, scale=1.0)
                acc = self.atmp.tile([128, CHUNK], BF16, tag="asm", name="asm")
                nc.vector.tensor_tensor(out=acc, in0=E[ch][c], in1=ssb,
                                        op=OP.mult)
                for i in range(n_in):
                    a_ps = self.p_mm.tile([128, CHUNK], F32, tag="mm", name="mm")
                    nc.tensor.matmul(
                        a_ps[0:D, :],
                        staged[i][0:D, (2 * c) * D : (2 * c + 1) * D],
                        E[ch][c][0:D, :], start=True, stop=True)
                    nc.tensor.matmul(
                        a_ps[D:128, :],
                        staged[i][D:128, (2 * c + 1) * D : (2 * c + 2) * D],
                        E[ch][c][D:128, :], start=True, stop=True)
                    j = 32 * (1 + i)
                    gb_ps = self.p_bc.tile([128, CHUNK], F32, tag="bc", name="bc")
                    nc.tensor.matmul(gb_ps, self.SEL24[c][j : j + 8, :],
                                     rr[j : j + 8, :],
                                     start=True, stop=True)
                    gsb = self.atmp.tile([128, CHUNK], BF16, tag="gbs", name="gbs")
                    nc.scalar.activation(out=gsb, in_=gb_ps, func=AF.Copy,
                                         bias=0.0, scale=1.0)
                    ai = self.atmp.tile([128, CHUNK], BF16, tag="asm", name="asm")
                    nc.vector.tensor_tensor(out=ai, in0=a_ps, in1=gsb,
                                            op=OP.mult)
                    nxt = self.outcp.tile([128, CHUNK], BF16, tag="outc", name="outc") \
                        if i == n_in - 1 else \
                        self.atmp.tile([128, CHUNK], BF16, tag="asm", name="asm")
                    nc.gpsimd.tensor_tensor(out=nxt, in0=acc, in1=ai, op=OP.add)
                    acc = nxt
                outc.append(acc)
            for m in range(FT):
                wps = self.proj_fm_psum(WO, outc, m)
                tt = self.wotp.tile([128, CHUNK], F32, tag="wot", name="wot")
                nc.scalar.activation(out=tt, in_=wps, func=AF.Identity,
                                     bias=bo_cols[:, m : m + 1], scale=1.0)
                xo = out_pool.tile([128, CHUNK], BF16, tag="resid", name="resid")
                nc.gpsimd.tensor_tensor(out=xo, in0=Xin[ch][m], in1=tt,
                                        op=OP.add)
                Xout[ch][m] = xo
        return Xout

    def ffn(self, Xin, w1name, w2name, B1, B2, out_pool, out_dtype=BF16,
            out_cb=None):
        """xs -> gelu(W1 xs + b1) -> W2 h + b2 -> residual add. Weights are
        loaded into phase-scoped pools."""
        nc, tc, I = self.nc, self.tc, self.I
        Xout = [[None] * FT for _ in range(NCH)]
        with tc.tile_pool(name=w1name, bufs=1) as wp, \
             tc.tile_pool(name="hp", bufs=IT) as hp, \
             tc.tile_pool(name="p_acc", bufs=2, space="PSUM") as p_acc:
            W1 = []
            for k in range(FT):
                t = wp.tile([128, INNER], BF16, tag=f"w1_{k}", name=f"w1_{k}")
                nc.sync.dma_start(
                    out=t, in_=I[w1name][128 * k : 128 * (k + 1), :])
                W1.append(t)
            W2 = []
            for k in range(IT):
                t = wp.tile([128, C], BF16, tag=f"w2_{k}", name=f"w2_{k}")
                nc.sync.dma_start(
                    out=t, in_=I[w2name][128 * k : 128 * (k + 1), :])
                W2.append(t)
            XS = self.ln_batch(Xin)
            for ch in range(NCH):
                xs = XS[ch]
                hs = []
                for k in range(IT):
                    hps = self.p_mm.tile([128, CHUNK], F32, tag="mm", name="mm")
                    for c in range(FT):
                        nc.tensor.matmul(hps, W1[c][:, 128 * k : 128 * (k + 1)],
                                         xs[c], start=(c == 0),
                                         stop=(c == FT - 1))
                    h = hp.tile([128, CHUNK], BF16, tag="h", name="h")
                    nc.scalar.activation(out=h, in_=hps, func=AF.Gelu_apprx_tanh,
                                         bias=B1[:, k : k + 1], scale=1.0)
                    hs.append(h)
                for m in range(FT):
                    acc = p_acc.tile([128, CHUNK], F32, tag="acc", name="acc")
                    for k in range(IT):
                        nc.tensor.matmul(acc, W2[k][:, 128 * m : 128 * (m + 1)],
                                         hs[k], start=(k == 0),
                                         stop=(k == IT - 1))
                    tt = self.wotp.tile([128, CHUNK], F32, tag="wot", name="wot")
                    nc.scalar.activation(out=tt, in_=acc, func=AF.Identity,
                                         bias=B2[:, m : m + 1], scale=1.0)
                    xo = out_pool.tile([128, CHUNK], out_dtype, tag="resid",
                                       name="resid")
                    nc.gpsimd.tensor_tensor(out=xo, in0=Xin[ch][m], in1=tt,
                                            op=OP.add)
                    Xout[ch][m] = xo
                    if out_cb is not None:
                        out_cb(ch, m, xo)
        return Xout

    # ---------------- main ----------------
    def run(self):
        nc, tc, I = self.nc, self.tc, self.I
        from contextlib import ExitStack

        with ExitStack() as ctx:
            const = ctx.enter_context(tc.tile_pool(name="const", bufs=1))
            self.resid = ctx.enter_context(tc.tile_pool(name="resid", bufs=36))
            self.xnp = ctx.enter_context(tc.tile_pool(name="xn", bufs=18))
            self.rows = ctx.enter_context(tc.tile_pool(name="rows", bufs=4))
            self.rowsb = ctx.enter_context(tc.tile_pool(name="rowsb", bufs=4))
            self.lntmp = ctx.enter_context(tc.tile_pool(name="lntmp", bufs=4))
            self.lnab = ctx.enter_context(tc.tile_pool(name="lnab", bufs=2))
            self.wotp = ctx.enter_context(tc.tile_pool(name="wot", bufs=2))
            self.kep = ctx.enter_context(tc.tile_pool(name="kep", bufs=2))
            self.kvp = ctx.enter_context(tc.tile_pool(name="kvp", bufs=3))
            self.smallp = ctx.enter_context(tc.tile_pool(name="smallp", bufs=4))
            self.recp = ctx.enter_context(tc.tile_pool(name="recp", bufs=2))
            self.recb = ctx.enter_context(tc.tile_pool(name="recb", bufs=4))
            self.atmp = ctx.enter_context(tc.tile_pool(name="atmp", bufs=3))
            self.outcp = ctx.enter_context(tc.tile_pool(name="outcp", bufs=5))
            dram = ctx.enter_context(tc.tile_pool(name="dram", bufs=1,
                                                  space="DRAM"))
            self.p_mm = ctx.enter_context(
                tc.tile_pool(name="p_mm", bufs=2, space="PSUM"))
            self.p_stats = ctx.enter_context(
                tc.tile_pool(name="p_stats", bufs=2, space="PSUM"))
            self.p_bc = ctx.enter_context(
                tc.tile_pool(name="p_bc", bufs=1, space="PSUM"))

            # ---------------- constants ----------------
            self.EPS = const.tile([1, 1], F32, tag="eps", name="eps")
            nc.vector.memset(self.EPS, LN_EPS)
            self.ONESC = const.tile([1, 128], BF16, tag="onesc", name="onesc")
            nc.sync.dma_start(out=self.ONESC, in_=I["ones_c"])
            self.ONESR = const.tile([128, 1], BF16, tag="onesr", name="onesr")
            nc.sync.dma_start(out=self.ONESR, in_=I["ones_r"])
            self.SEL24 = []
            for c in range(FT):
                s = const.tile([72, 128], BF16, tag=f"sel{c}", name=f"sel{c}")
                nc.sync.dma_start(out=s, in_=I["sel24"][c])
                self.SEL24.append(s)

            def cols_tile(name, nt):
                t = const.tile([128, nt], F32, tag=name)
                nc.sync.dma_start(out=t, in_=I[name])
                return t

            BQ = cols_tile("bq_c", FT)
            BO = cols_tile("bo_c", FT)
            SAQ = cols_tile("saq_c", FT)
            SAO = cols_tile("sao_c", FT)
            F1B1 = cols_tile("f1b1_c", IT)
            F1B2 = cols_tile("f1b2_c", FT)
            F2B1 = cols_tile("f2b1_c", IT)
            F2B2 = cols_tile("f2b2_c", FT)

            def row_tile(apslice, tag):
                t = const.tile([1, C], BF16, tag=tag)
                nc.sync.dma_start(out=t, in_=apslice)
                return t

            BKR = [row_tile(I["bk_r"][i], f"bkr{i}") for i in range(NIN)]
            BVR = [row_tile(I["bv_r"][i], f"bvr{i}") for i in range(NIN)]
            SAKR = row_tile(I["sak_r"], "sakr")
            SAVR = row_tile(I["sav_r"], "savr")

            # ---------------- attn weights (scoped; freed before FFN2) ----
            wpool_ctx = tc.tile_pool(name="weights", bufs=1)
            wpool = wpool_ctx.__enter__()
            WQ = self.load_w512(I["wq"], wpool, "wq")
            WO = self.load_w512(I["wo"], wpool, "wo")
            SWQ = self.load_w512(I["saq"], wpool, "saq")
            SWK = self.load_w512(I["sak"], wpool, "sak")
            SWV = self.load_w512(I["sav"], wpool, "sav")
            SWO = self.load_w512(I["sao"], wpool, "sao")
            WK = [self.load_w512(I["wk"][i], wpool, f"wk{i}") for i in range(NIN)]
            WV = [self.load_w512(I["wv"][i], wpool, f"wv{i}") for i in range(NIN)]

            # ---------------- residual load ----------------
            X = [[self.resid.tile([128, CHUNK], BF16, tag="resid", name="resid")
                  for _ in range(FT)] for _ in range(NCH)]
            for ch in range(NCH):
                for c in range(FT):
                    nc.sync.dma_start(
                        out=X[ch][c],
                        in_=I["xT"][128 * c : 128 * (c + 1),
                                    CHUNK * ch : CHUNK * (ch + 1)])

            cc_in = dram.tile([NIN, D + 1, C], F32, tag="cc_ca_in", name="cc_ca_in")
            cc_out = dram.tile([NIN, D + 1, C], F32, tag="cc_ca_out", name="cc_ca_out")
            cc2_in = dram.tile([D + 1, C], F32, tag="cc_sa_in", name="cc_sa_in")
            cc2_out = dram.tile([D + 1, C], F32, tag="cc_sa_out", name="cc_sa_out")

            # ============ phase A: CA kv side ============
            with tc.tile_pool(name="ysp", bufs=10) as ysp, \
                 tc.tile_pool(name="ctxsb", bufs=1) as ctxsbp, \
                 tc.tile_pool(name="p_ctx", bufs=1, space="PSUM") as p_ctx:
                KSA = [ctxsbp.tile([1, C], F32, tag=f"ksacc{i}",
                                   name=f"ksacc{i}") for i in range(NIN)]
                CTX_PS = [p_ctx.tile([D, C], F32, tag=f"ctx{i}", name=f"ctx{i}")
                          for i in range(NIN)]
                for i in range(NIN):
                    YS = {}
                    for ch in range(NCH):
                        yt = []
                        for c in range(FT):
                            y = ysp.tile([128, CHUNK], BF16, tag="ys", name="ys")
                            nc.sync.dma_start(
                                out=y,
                                in_=I["ysT"][i, 128 * c : 128 * (c + 1),
                                             CHUNK * ch : CHUNK * (ch + 1)])
                            yt.append(y)
                        YS[ch] = self.ln_batch({ch: yt}, chunks=[ch])[ch]
                    self.kv_side(YS, WK[i], WV[i], BKR[i], BVR[i],
                                 CTX_PS[i], KSA[i])
                for i in range(NIN):
                    ct = ctxsbp.tile([D, C], F32, tag=f"ctc{i}", name=f"ctc{i}")
                    nc.vector.tensor_copy(ct, CTX_PS[i])
                    nc.sync.dma_start(out=cc_in[i, 0:D, :], in_=ct)
                    nc.sync.dma_start(out=cc_in[i, D : D + 1, :], in_=KSA[i])
            nc.gpsimd.collective_compute(
                "AllReduce", OP.add, replica_groups=GROUPS,
                ins=[cc_in[:].opt()], outs=[cc_out[:].opt()])

            # ============ q side (overlaps the AllReduce) ============
            with tc.tile_pool(name="E1", bufs=16) as epool1:
                XS1 = self.ln_batch(X)
                E = self.front_E(XS1, WQ, BQ, epool1)

                # ============ CA back + FFN1 ============
                with tc.tile_pool(name="stag1", bufs=1) as staging1:
                    staged, sgt = self.build_staged(cc_out, NIN, staging1)
                    X1 = self.attn_back(X, E, staged, sgt, NIN, WO, BO,
                                        self.resid)
            X2 = self.ffn(X1, "f1w1", "f1w2", F1B1, F1B2, self.resid)

            # ============ phase C: SA (shared ln4) ============
            XS4 = self.ln_batch(X2)
            with tc.tile_pool(name="ctxsb2", bufs=1) as ctxsbp2, \
                 tc.tile_pool(name="p_ctx2", bufs=1, space="PSUM") as p_ctx2:
                KSA2 = ctxsbp2.tile([1, C], F32, tag="ksacc2", name="ksacc2")
                ctx2_ps = p_ctx2.tile([D, C], F32, tag="ctx2", name="ctx2")
                self.kv_side(XS4, SWK, SWV, SAKR, SAVR, ctx2_ps, KSA2)
                ct2 = ctxsbp2.tile([D, C], F32, tag="ctc2", name="ctc2")
                nc.vector.tensor_copy(ct2, ctx2_ps)
                nc.sync.dma_start(out=cc2_in[0:D, :], in_=ct2)
                nc.sync.dma_start(out=cc2_in[D : D + 1, :], in_=KSA2)
            nc.gpsimd.collective_compute(
                "AllReduce", OP.add, replica_groups=GROUPS,
                ins=[cc2_in[:].opt()], outs=[cc2_out[:].opt()])

            with tc.tile_pool(name="E2", bufs=16) as epool2:
                E2 = self.front_E(XS4, SWQ, SAQ, epool2)
                with tc.tile_pool(name="stag2", bufs=1) as staging2:
                    staged2, sgt2 = self.build_staged(cc2_out, 1, staging2)
                    X3 = self.attn_back(X2, E2, staged2, sgt2, 1, SWO, SAO,
                                        self.resid)

            wpool_ctx.__exit__(None, None, None)

            # ============ FFN 2 (fp32 residual out) ============
            def emit_out(ch, m, xo):
                nc.sync.dma_start(
                    out=self.out_t[128 * m : 128 * (m + 1),
                                   CHUNK * ch : CHUNK * (ch + 1)],
                    in_=xo)

            with tc.tile_pool(name="fout", bufs=5) as fout:
                self.ffn(X3, "f2w1", "f2w2", F2B1, F2B2, fout,
                         out_dtype=F32, out_cb=emit_out)


# ---------------------------------------------------------------------------
# host side
# ---------------------------------------------------------------------------
_PROGRAM = None
LAST_RESULTS = None


def _cols(v, nt):
    return np.ascontiguousarray(np.asarray(v, np.float32).reshape(nt, 128).T)


def _host_consts():
    import ml_dtypes
    bf = ml_dtypes.bfloat16
    sgbase = np.zeros((FT, 128, 72), np.float32)
    sel24 = np.zeros((FT, 72, 128), np.float32)
    for c in range(FT):
        for p in range(128):
            h = 2 * c + (1 if p >= 64 else 0)
            sgbase[c, p, h] = 1.0
            # pad columns replicate the S one-hot so the batched
            # reciprocal never sees zeros in unused group rows
            for col in range(72):
                if col % 32 >= 8:
                    sgbase[c, p, col] = 1.0 if col % 8 == h else 0.0
            for j in range(3):
                sel24[c, j * 32 + h, p] = 1.0
    return {
        "ones_c": np.ones((1, 128), bf),
        "ones_r": np.ones((128, 1), bf),
        "sgbase": sgbase.astype(bf),
        "sel24": sel24.astype(bf),
    }


def _make_in_maps(inputs):
    import ml_dtypes
    bf = ml_dtypes.bfloat16
    f = lambda k: np.asarray(inputs[k], np.float32)

    def fold_w(w, g):
        # upload layout W.T [in, out], rows scaled by LN gain
        return np.ascontiguousarray(w.T * g[:, None]).astype(bf)

    def fold_bc(w, b_ln, bias, nt):
        # folded output bias cb = b_ln @ W.T + bias, as column tiles
        return _cols(b_ln @ w.T + bias, nt)

    g1, b1 = f("ln1_g"), f("ln1_b")
    g2, b2 = f("ln2_g"), f("ln2_b")
    g3, b3 = f("ln3_g"), f("ln3_b")
    g4, b4 = f("ln4_g"), f("ln4_b")
    g5, b5 = f("ln5_g"), f("ln5_b")

    wk = f("ca_wk")
    wv = f("ca_wv")
    shared = {
        "wq": fold_w(f("ca_wq"), g1),
        "bq_c": fold_bc(f("ca_wq"), b1, f("ca_bq"), FT),
        "wo": np.ascontiguousarray(f("ca_wo").T).astype(bf),
        "bo_c": _cols(f("ca_bo"), FT),
        "wk": np.stack([fold_w(wk[i], g2[i]) for i in range(NIN)]),
        "wv": np.stack([fold_w(wv[i], g2[i]) for i in range(NIN)]),
        "bk_r": np.stack([(b2[i] @ wk[i].T + f("ca_bk")[i]).reshape(1, C)
                          for i in range(NIN)]).astype(bf),
        "bv_r": np.stack([(b2[i] @ wv[i].T + f("ca_bv")[i]).reshape(1, C)
                          for i in range(NIN)]).astype(bf),
        "saq": fold_w(f("sa_wq"), g4),
        "saq_c": fold_bc(f("sa_wq"), b4, f("sa_bq"), FT),
        "sak": fold_w(f("sa_wk"), g4),
        "sak_r": (b4 @ f("sa_wk").T + f("sa_bk")).reshape(1, C).astype(bf),
        "sav": fold_w(f("sa_wv"), g4),
        "sav_r": (b4 @ f("sa_wv").T + f("sa_bv")).reshape(1, C).astype(bf),
        "sao": np.ascontiguousarray(f("sa_wo").T).astype(bf),
        "sao_c": _cols(f("sa_bo"), FT),
        "f1w1": fold_w(f("ffn1_w1"), g3),
        "f1b1_c": fold_bc(f("ffn1_w1"), b3, f("ffn1_b1"), IT),
        "f1w2": np.ascontiguousarray(f("ffn1_w2").T).astype(bf),
        "f1b2_c": _cols(f("ffn1_b2"), FT),
        "f2w1": fold_w(f("ffn2_w1"), g5),
        "f2b1_c": fold_bc(f("ffn2_w1"), b5, f("ffn2_b1"), IT),
        "f2w2": np.ascontiguousarray(f("ffn2_w2").T).astype(bf),
        "f2b2_c": _cols(f("ffn2_b2"), FT),
    }
    shared.update(_host_consts())

    x = f("x")
    ys = f("ys")
    in_maps = []
    for core in range(N_CORES):
        b, half = core // 2, core % 2
        lo, hi = half * NTOK, (half + 1) * NTOK
        m = dict(shared)
        m["xT"] = np.ascontiguousarray(x[b, lo:hi, :].T).astype(bf)
        m["ysT"] = np.ascontiguousarray(
            ys[:, b, lo:hi, :].transpose(0, 2, 1)).astype(bf)
        in_maps.append(m)
    return in_maps


def kernel(**inputs):
    global _PROGRAM, LAST_RESULTS
    if _PROGRAM is None:
        _PROGRAM = _build_program()
    nc = _PROGRAM
    in_maps = _make_in_maps(inputs)

    trace = os.environ.get("BASS_TRACE", "") not in ("", "0")
    res = run_bass_kernel_spmd(nc, in_maps, core_ids=list(range(N_CORES)),
                               trace=trace)
    LAST_RESULTS = res

    out = np.empty((B, T, C), np.float32)
    for core in range(N_CORES):
        b, half = core // 2, core % 2
        lo, hi = half * NTOK, (half + 1) * NTOK
        out[b, lo:hi, :] = res.results[core]["outT"].T
    return out


# revision 10
# speedup vs baseline: 1.6357x; 1.3091x over previous
"""Trainium2 Bass kernel for nn_CrossAttentionBlock (B=4, T=4096, C=512, H=8,
INNER=2048, NIN=2) on 8 NeuronCores.

Sharding: core c handles batch b=c//2, token half h=c%2 (2048 tokens each).
All per-token math is local; the only cross-core coupling is the linear-
attention context (ctx = k^T v, [H,64,64] per batch) and k_sum, reduced with
pair-wise AllReduces (cores 2b and 2b+1).

v2 design (all-bf16 matmul datapath):
- Every matmul runs in bf16 (weights pre-cast host-side, activations written
  as bf16 by the producing Act/DVE/Pool op). PSUM accumulation stays fp32.
  This avoids the throttled fp32_mode=HIGH PE path and halves LDWEIGHTS.
- LayerNorm affine (g, b) is folded into the consuming projection weights on
  the host (rows scaled by g, b folded into the output bias), so on-chip LN
  only computes per-token a=rsqrt(var+eps) and mean, then xs = x*aB - amB.
- Per-token LN stats use fast fp32 reciprocal_approx_fast (never the 3.3us
  fp32r-output reciprocal path); rsqrt = recip(sqrt()).
- Loops are batched by activation function (Sqrt / Exp / Gelu) to minimize
  1.28us ACT_TABLE_LOADs, and chunks pipeline across engines.
- Elementwise work is spread across DVE and the otherwise-idle Pool engine.
- k/v sides run before the q sides so the q projections + exp overlap the
  AllReduce; SG/BD stationaries are built with compute-engine copies from a
  staged bf16 ctx tile instead of many small scatter DMAs.
- All weights live in SBUF for the whole kernel (no re-streaming).
"""
import os
import numpy as np

import concourse.bass as bass
import concourse.tile as tile
from concourse import mybir
from concourse.vector_clock import ScopedClock
from concourse.bass_utils import run_bass_kernel_spmd

F32 = mybir.dt.float32
BF16 = mybir.dt.bfloat16
AF = mybir.ActivationFunctionType
OP = mybir.AluOpType

B, T, C, H, D, INNER, NIN = 4, 4096, 512, 8, 64, 2048, 2
N_CORES = 8
NTOK = 2048          # tokens per core
CHUNK = 512          # tokens per chunk
NCH = NTOK // CHUNK  # 4 chunks
FT = C // 128        # 4 feature tiles
IT = INNER // 128    # 16 inner tiles
LN_EPS = 1e-5
GROUPS = [[0, 1], [2, 3], [4, 5], [6, 7]]

_split_counter = [0]


def _split_multi_waits(nc):
    """This walrus build only supports one sync-wait per instruction; move
    extra waits onto same-engine NoOps placed immediately before."""
    for f in nc.m.functions:
        for blk in f.blocks:
            out = []
            changed = False
            for inst in blk.instructions:
                si = inst.sync_info
                if si is not None and si.on_wait and len(si.on_wait) > 1:
                    waits = list(si.on_wait)
                    for w in waits[:-1]:
                        _split_counter[0] += 1
                        nop = mybir.InstNoOp(
                            name=f"I-waitsplit-{_split_counter[0]}", ins=[], outs=[]
                        )
                        nop.engine = inst.engine
                        nop.sync_info = mybir.SyncInfo(on_wait=[w], on_update=[])
                        out.append(nop)
                    si.on_wait = waits[-1:]
                    inst.sync_info = si
                    changed = True
                out.append(inst)
            if changed:
                blk.instructions = out


class _TC(tile.TileContext):
    def _drain_and_barrier(self, tick_clock, wait_clock):
        drain_inst = self.nc.sync.drain()
        wait_clock.add_sem_waits(
            drain_inst.ins, ScopedClock({None: tick_clock.global_clock})
        )
        si = drain_inst.ins.sync_info
        if si is not None and si.on_wait and len(si.on_wait) > 1:
            waits = list(si.on_wait)
            si.on_wait = waits[:1]
            drain_inst.ins.sync_info = si
            for i in range(1, len(waits)):
                extra = self.nc.sync.drain()
                esi = extra.ins.sync_info
                if esi is None:
                    extra.ins.sync_info = mybir.SyncInfo(
                        on_wait=waits[i : i + 1], on_update=[]
                    )
                else:
                    esi.on_wait = waits[i : i + 1]
                    extra.ins.sync_info = esi
        self.nc.all_engine_barrier()
        assert self.sems is not None
        popped = self.nc._tile_sem_poison_stack.pop()
        assert popped is self._sem_poison
        self.nc.clear_and_free_semaphores(list(self.sems.allocated().values()))
        self.nc.all_engine_barrier()


def _build_program(split=True):
    nc = bass.Bass("TRN2", target_bir_lowering=False, debug=False, num_devices=N_CORES)
    I = {}

    def di(name, shape, dt=BF16):
        I[name] = nc.dram_tensor(name, list(shape), dt, kind="ExternalInput").ap()

    di("xT", [C, NTOK])
    di("ysT", [NIN, C, NTOK])
    for w in ["wq", "wo", "saq", "sak", "sav", "sao"]:
        di(w, [C, C])
    di("wk", [NIN, C, C])
    di("wv", [NIN, C, C])
    di("f1w1", [C, INNER])
    di("f1w2", [INNER, C])
    di("f2w1", [C, INNER])
    di("f2w2", [INNER, C])
    for bname in ["bq_c", "bo_c", "saq_c", "sao_c", "f1b2_c", "f2b2_c"]:
        di(bname, [128, FT], F32)
    di("f1b1_c", [128, IT], F32)
    di("f2b1_c", [128, IT], F32)
    di("expbB", [NIN, 128, C])
    di("bvB", [NIN, 128, C])
    di("saexpbB", [128, C])
    di("sabvB", [128, C])
    di("ones_c", [1, 128])
    di("ones_r", [128, 1])
    di("sgbase", [FT, 128, 72])
    di("sel24", [FT, 72, 128])

    out_t = nc.dram_tensor("outT", [C, NTOK], F32, kind="ExternalOutput").ap()

    with _TC(nc) as tc:
        with nc.allow_low_precision(reason="bf16 datapath, tolerance 2e-2"):
            _Emitter(nc, tc, I, out_t).run()
    if split:
        _split_multi_waits(nc)
    return nc


class _Emitter:
    def __init__(self, nc, tc, I, out_t):
        self.nc, self.tc, self.I, self.out_t = nc, tc, I, out_t

    # ---------------- layer norm (folded affine) ----------------
    def act_direct(self, out, in_, func, bias=0.0, scale=1.0):
        """Emit an InstActivation with a func the wrapper refuses
        (Rsqrt/Reciprocal). Accuracy is table-interp grade (~1e-3), fine
        for the 2e-2 gate. Walrus inserts the table load from the final
        func, so mutating post-emission is safe."""
        inst = self.nc.scalar.activation(out=out, in_=in_, func=AF.Sqrt,
                                         bias=bias, scale=scale)
        inst.ins.func = func
        return inst

    def ln_stats(self, x):
        """Stat matmuls + squares for one chunk -> (s_ps, q_ps)."""
        nc = self.nc
        s_ps = self.p_stats.tile([1, CHUNK], F32, tag="stats", name="stats")
        for k in range(FT):
            nc.tensor.matmul(s_ps, self.ONESR, x[k],
                             start=(k == 0), stop=(k == FT - 1))
        xsq = []
        for k in range(FT):
            sq = self.lntmp.tile([128, CHUNK], BF16, tag="xsq", name="xsq")
            nc.scalar.activation(out=sq, in_=x[k], func=AF.Square)
            xsq.append(sq)
        q_ps = self.p_stats.tile([1, CHUNK], F32, tag="stats", name="stats")
        for k in range(FT):
            nc.tensor.matmul(q_ps, self.ONESR, xsq[k],
                             start=(k == 0), stop=(k == FT - 1))
        return s_ps, q_ps

    def ln_finish(self, x, s_ps, q_ps):
        """Row chain + broadcasts + xs tiles for one chunk."""
        nc = self.nc
        mrow = self.rows.tile([1, CHUNK], F32, tag="rows", name="rows")
        nc.vector.tensor_scalar(out=mrow, in0=s_ps, scalar1=1.0 / C,
                                scalar2=None, op0=OP.mult)
        s2 = self.rows.tile([1, CHUNK], F32, tag="rows", name="rows")
        nc.vector.tensor_tensor(out=s2, in0=mrow, in1=mrow, op=OP.mult)
        v = self.rows.tile([1, CHUNK], F32, tag="rows", name="rows")
        nc.vector.scalar_tensor_tensor(out=v, in0=q_ps, scalar=1.0 / C,
                                       in1=s2, op0=OP.mult, op1=OP.subtract)
        arow = self.rowsb.tile([1, CHUNK], BF16, tag="rowsb", name="rowsb")
        self.act_direct(arow, v, AF.Rsqrt, bias=self.EPS, scale=1.0)
        am = self.rowsb.tile([1, CHUNK], BF16, tag="rowsb", name="rowsb")
        nc.vector.tensor_tensor(out=am, in0=arow, in1=mrow, op=OP.mult)
        bc_ps = self.p_bc.tile([128, CHUNK], F32, tag="bc", name="bc")
        nc.tensor.matmul(bc_ps, self.ONESC, arow, start=True, stop=True)
        bc_ps2 = self.p_bc.tile([128, CHUNK], F32, tag="bc", name="bc")
        nc.tensor.matmul(bc_ps2, self.ONESC, am, start=True, stop=True)
        aB = self.lnab.tile([128, CHUNK], BF16, tag="aB", name="aB")
        nc.scalar.activation(out=aB, in_=bc_ps, func=AF.Copy, bias=0.0,
                             scale=1.0)
        amB = self.lnab.tile([128, CHUNK], BF16, tag="amB", name="amB")
        nc.vector.tensor_copy(amB, bc_ps2)
        xs = []
        for k in range(FT):
            t1 = self.lntmp.tile([128, CHUNK], BF16, tag="lnt", name="lnt")
            nc.vector.tensor_tensor(out=t1, in0=x[k], in1=aB, op=OP.mult)
            xk = self.xnp.tile([128, CHUNK], BF16, tag="xn", name="xn")
            nc.gpsimd.tensor_tensor(out=xk, in0=t1, in1=amB, op=OP.subtract)
            xs.append(xk)
        return xs

    def ln_batch(self, Xin, chunks=None):
        """LN with one-chunk lookahead: chunk N+1's stat matmuls are
        emitted before chunk N's broadcasts so the PE never stalls on the
        row chain."""
        if chunks is None:
            chunks = range(NCH)
        XS = {}
        pend = None
        for ch in chunks:
            st = self.ln_stats(Xin[ch])
            if pend is not None:
                pch, px, ps, pq = pend
                XS[pch] = self.ln_finish(px, ps, pq)
            pend = (ch, Xin[ch], st[0], st[1])
        pch, px, ps, pq = pend
        XS[pch] = self.ln_finish(px, ps, pq)
        return XS

    # ---------------- matmul helpers ----------------
    def proj_fm_psum(self, w_tiles, xs, m):
        ps = self.p_mm.tile([128, CHUNK], F32, tag="mm", name="mm")
        for k in range(FT):
            self.nc.tensor.matmul(ps, w_tiles[k][:, 128 * m : 128 * (m + 1)],
                                  xs[k], start=(k == 0), stop=(k == FT - 1))
        return ps

    def proj_tm_psum(self, w_tiles, xs, t):
        ps = self.p_mm.tile([128, CHUNK], F32, tag="mm", name="mm")
        for k in range(FT):
            self.nc.tensor.matmul(ps, xs[k][:, 128 * t : 128 * (t + 1)],
                                  w_tiles[k], start=(k == 0),
                                  stop=(k == FT - 1))
        return ps

    def load_w512(self, ap, pool, tag):
        tiles = []
        for k in range(FT):
            t = pool.tile([128, C], BF16, tag=f"{tag}{k}", name=f"{tag}{k}")
            self.nc.sync.dma_start(out=t, in_=ap[128 * k : 128 * (k + 1), :])
            tiles.append(t)
        return tiles

    # ---------------- attention pieces ----------------
    def kv_side(self, xs_by_ch, WK, WV, expbB, bvB, ctx_ps, ks_acc):
        """k/v projections (token-major, bias via broadcast consts),
        k-softmax over d, head-pair ctx accumulation ([128,C] psum in raw
        pair layout), ks accumulated into SBUF."""
        nc = self.nc
        first = [True]
        for ch in range(NCH):
            xs = xs_by_ch[ch]
            ks_ps = self.p_stats.tile([1, C], F32, tag="stats", name="stats")
            for t in range(FT):
                kps = self.proj_tm_psum(WK, xs, t)
                kE = self.kep.tile([128, C], BF16, tag="kE", name="kE")
                nc.scalar.activation(out=kE, in_=kps, func=AF.Exp)
                kEb = self.kep.tile([128, C], BF16, tag="kEb", name="kEb")
                nc.vector.tensor_tensor(out=kEb, in0=kE, in1=expbB, op=OP.mult)
                ssum = self.smallp.tile([128, H], F32, tag="ssum", name="ssum")
                nc.vector.tensor_reduce(
                    out=ssum, in_=kEb.rearrange("p (h d) -> p h d", d=D),
                    axis=mybir.AxisListType.X, op=OP.add)
                rsum = self.smallp.tile([128, H], F32, tag="rsum", name="rsum")
                nc.vector.reciprocal(out=rsum, in_=ssum)
                kn = self.kvp.tile([128, C], BF16, tag="kn", name="kn")
                for h in range(H):
                    nc.vector.tensor_scalar(
                        out=kn[:, D * h : D * (h + 1)],
                        in0=kEb[:, D * h : D * (h + 1)],
                        scalar1=rsum[:, h : h + 1], scalar2=None, op0=OP.mult)
                vps = self.proj_tm_psum(WV, xs, t)
                vn = self.kvp.tile([128, C], BF16, tag="vn", name="vn")
                nc.vector.tensor_tensor(out=vn, in0=vps, in1=bvB, op=OP.add)
                for j in range(FT):
                    nc.tensor.matmul(
                        ctx_ps[:, 128 * j : 128 * (j + 1)],
                        kn[:, 128 * j : 128 * (j + 1)],
                        vn[:, 128 * j : 128 * (j + 1)],
                        start=(first[0] and j == 0),
                        stop=(ch == NCH - 1 and t == FT - 1 and j == FT - 1))
                nc.tensor.matmul(ks_ps, self.ONESR, kn, start=(t == 0),
                                 stop=(t == FT - 1))
                first[0] = False
            if ch == 0:
                nc.vector.tensor_copy(ks_acc, ks_ps)
            else:
                nc.vector.tensor_tensor(out=ks_acc, in0=ks_acc, in1=ks_ps,
                                        op=OP.add)

    def front_E(self, xs_by_ch, W, b_cols, epool):
        """q projection + exp -> E tiles (bf16). Act func: Exp."""
        nc = self.nc
        E = [[None] * FT for _ in range(NCH)]
        for ch in range(NCH):
            for m in range(FT):
                ps = self.proj_fm_psum(W, xs_by_ch[ch], m)
                e = epool.tile([128, CHUNK], BF16, tag="E", name="E")
                nc.scalar.activation(out=e, in_=ps, func=AF.Exp,
                                     bias=b_cols[:, m : m + 1], scale=1.0)
                E[ch][m] = e
        return E

    def build_staged(self, cc_ap, n_in, staging):
        """From AllReduced cc ([n,129,C] or [129,C] fp32 DRAM): per input a
        [128,C] bf16 tile in the raw head-pair layout (even heads on
        partitions 0:63, odd on 64:127), a transposed ksum [128,FT] fp32
        tile, and sg stationaries [128,ncols] bf16 per feature tile."""
        nc, I = self.nc, self.I
        cc = (lambda i: cc_ap[i]) if n_in > 1 else (lambda i: cc_ap)
        ncols = 32 * n_in + 8
        staged, sgt = [], []
        for c in range(FT):
            sg = staging.tile([128, ncols], BF16, tag=f"sg{c}", name=f"sg{c}")
            nc.sync.dma_start(out=sg, in_=I["sgbase"][c][:, 0:ncols])
            sgt.append(sg)
        for i in range(n_in):
            st = staging.tile([128, C], F32, tag=f"st{i}", name=f"st{i}")
            nc.sync.dma_start(out=st, in_=cc(i)[0:128, :])
            stb = staging.tile([128, C], BF16, tag=f"stb{i}", name=f"stb{i}")
            nc.scalar.activation(out=stb, in_=st, func=AF.Copy, bias=0.0,
                                 scale=1.0)
            kst = staging.tile([128, FT], F32, tag=f"kst{i}", name=f"kst{i}")
            nc.sync.dma_start(
                out=kst,
                in_=cc(i)[128 : 129, :].rearrange("o (a p) -> p (o a)", p=128))
            for c in range(FT):
                col = 32 * (1 + i) + 2 * c
                nc.scalar.activation(
                    out=sgt[c][0:D, col : col + 1], in_=kst[0:D, c : c + 1],
                    func=AF.Copy, bias=0.0, scale=1.0)
                nc.scalar.activation(
                    out=sgt[c][D:128, col + 1 : col + 2],
                    in_=kst[D:128, c : c + 1],
                    func=AF.Copy, bias=0.0, scale=1.0)
            staged.append(stb)
        return staged, sgt

    def attn_back(self, Xin, E, staged, sgt, n_in, WO, bo_cols, out_pool):
        """Denominators (single M=8+8n SG matmul), reciprocal, broadcasts,
        per-head ctx apply (split-K from staged bf16), assembly, wo
        projection + residual."""
        nc, tc = self.nc, self.tc
        ncols = 32 * n_in + 8
        Xout = [[None] * FT for _ in range(NCH)]
        # --- all denominators + reciprocals (no act tables) ---
        RR = []
        p_sg_ctx = tc.tile_pool(name="p_sg", bufs=2, space="PSUM")
        p_sg = p_sg_ctx.__enter__()
        for ch in range(NCH):
            g_ps = p_sg.tile([ncols, CHUNK], F32, tag="sg", name="sg")
            for c in range(FT):
                nc.tensor.matmul(g_ps, sgt[c][:, 0:ncols], E[ch][c],
                                 start=(c == 0), stop=(c == FT - 1))
            rr = self.recb.tile([ncols, CHUNK], BF16, tag="recb", name="recb")
            self.act_direct(rr, g_ps, AF.Reciprocal)
            RR.append(rr)
        p_sg_ctx.__exit__(None, None, None)
        # --- apply + assemble + project ---
        for ch in range(NCH):
            rr = RR[ch]
            outc = []
            for c in range(FT):
                sb_ps = self.p_bc.tile([128, CHUNK], F32, tag="bc", name="bc")
                nc.tensor.matmul(sb_ps, self.SEL24[c][0:8, :], rr[0:8, :],
                                 start=True, stop=True)
                ssb = self.atmp.tile([128, CHUNK], BF16, tag="gbs", name="gbs")
                nc.scalar.activation(out=ssb, in_=sb_ps, func=AF.Copy,
                                     bias=0.0, scale=1.0)
                acc = self.atmp.tile([128, CHUNK], BF16, tag="asm", name="asm")
                nc.vector.tensor_tensor(out=acc, in0=E[ch][c], in1=ssb,
                                        op=OP.mult)
                for i in range(n_in):
                    a_ps = self.p_mm.tile([128, CHUNK], F32, tag="mm", name="mm")
                    nc.tensor.matmul(
                        a_ps[0:D, :],
                        staged[i][0:D, 128 * c : 128 * c + D],
                        E[ch][c][0:D, :], start=True, stop=True)
                    nc.tensor.matmul(
                        a_ps[D:128, :],
                        staged[i][D:128, 128 * c + D : 128 * (c + 1)],
                        E[ch][c][D:128, :], start=True, stop=True)
                    j = 32 * (1 + i)
                    gb_ps = self.p_bc.tile([128, CHUNK], F32, tag="bc", name="bc")
                    nc.tensor.matmul(gb_ps, self.SEL24[c][j : j + 8, :],
                                     rr[j : j + 8, :],
                                     start=True, stop=True)
                    gsb = self.atmp.tile([128, CHUNK], BF16, tag="gbs", name="gbs")
                    nc.scalar.activation(out=gsb, in_=gb_ps, func=AF.Copy,
                                         bias=0.0, scale=1.0)
                    ai = self.atmp.tile([128, CHUNK], BF16, tag="asm", name="asm")
                    nc.vector.tensor_tensor(out=ai, in0=a_ps, in1=gsb,
                                            op=OP.mult)
                    nxt = self.outcp.tile([128, CHUNK], BF16, tag="outc", name="outc") \
                        if i == n_in - 1 else \
                        self.atmp.tile([128, CHUNK], BF16, tag="asm", name="asm")
                    nc.vector.tensor_tensor(out=nxt, in0=acc, in1=ai, op=OP.add)
                    acc = nxt
                outc.append(acc)
            for m in range(FT):
                wps = self.proj_fm_psum(WO, outc, m)
                tt = self.wotp.tile([128, CHUNK], F32, tag="wot", name="wot")
                nc.scalar.activation(out=tt, in_=wps, func=AF.Identity,
                                     bias=bo_cols[:, m : m + 1], scale=1.0)
                xo = out_pool.tile([128, CHUNK], BF16, tag="resid", name="resid")
                nc.gpsimd.tensor_tensor(out=xo, in0=Xin[ch][m], in1=tt,
                                        op=OP.add)
                Xout[ch][m] = xo
        return Xout

    def ffn(self, Xin, w1name, w2name, B1, B2, out_pool, out_dtype=BF16,
            out_cb=None):
        """xs -> gelu(W1 xs + b1) -> W2 h + b2 -> residual add. Weights are
        loaded into phase-scoped pools."""
        nc, tc, I = self.nc, self.tc, self.I
        Xout = [[None] * FT for _ in range(NCH)]
        with tc.tile_pool(name=w1name, bufs=1) as wp, \
             tc.tile_pool(name="hp", bufs=IT) as hp, \
             tc.tile_pool(name="p_acc", bufs=2, space="PSUM") as p_acc:
            W1 = []
            for k in range(FT):
                t = wp.tile([128, INNER], BF16, tag=f"w1_{k}", name=f"w1_{k}")
                nc.sync.dma_start(
                    out=t, in_=I[w1name][128 * k : 128 * (k + 1), :])
                W1.append(t)
            W2 = []
            for k in range(IT):
                t = wp.tile([128, C], BF16, tag=f"w2_{k}", name=f"w2_{k}")
                nc.sync.dma_start(
                    out=t, in_=I[w2name][128 * k : 128 * (k + 1), :])
                W2.append(t)
            XS = self.ln_batch(Xin)
            for ch in range(NCH):
                xs = XS[ch]
                hs = []
                for k in range(IT):
                    hps = self.p_mm.tile([128, CHUNK], F32, tag="mm", name="mm")
                    for c in range(FT):
                        nc.tensor.matmul(hps, W1[c][:, 128 * k : 128 * (k + 1)],
                                         xs[c], start=(c == 0),
                                         stop=(c == FT - 1))
                    h = hp.tile([128, CHUNK], BF16, tag="h", name="h")
                    nc.scalar.activation(out=h, in_=hps, func=AF.Gelu_apprx_tanh,
                                         bias=B1[:, k : k + 1], scale=1.0)
                    hs.append(h)
                for m in range(FT):
                    acc = p_acc.tile([128, CHUNK], F32, tag="acc", name="acc")
                    for k in range(IT):
                        nc.tensor.matmul(acc, W2[k][:, 128 * m : 128 * (m + 1)],
                                         hs[k], start=(k == 0),
                                         stop=(k == IT - 1))
                    tt = self.wotp.tile([128, CHUNK], F32, tag="wot", name="wot")
                    nc.scalar.activation(out=tt, in_=acc, func=AF.Identity,
                                         bias=B2[:, m : m + 1], scale=1.0)
                    xo = out_pool.tile([128, CHUNK], out_dtype, tag="resid",
                                       name="resid")
                    nc.gpsimd.tensor_tensor(out=xo, in0=Xin[ch][m], in1=tt,
                                            op=OP.add)
                    Xout[ch][m] = xo
                    if out_cb is not None:
                        out_cb(ch, m, xo)
        return Xout

    # ---------------- main ----------------
    def run(self):
        nc, tc, I = self.nc, self.tc, self.I
        from contextlib import ExitStack

        with ExitStack() as ctx:
            const = ctx.enter_context(tc.tile_pool(name="const", bufs=1))
            self.resid = ctx.enter_context(tc.tile_pool(name="resid", bufs=36))
            self.xnp = ctx.enter_context(tc.tile_pool(name="xn", bufs=18))
            self.rows = ctx.enter_context(tc.tile_pool(name="rows", bufs=4))
            self.rowsb = ctx.enter_context(tc.tile_pool(name="rowsb", bufs=4))
            self.lntmp = ctx.enter_context(tc.tile_pool(name="lntmp", bufs=4))
            self.lnab = ctx.enter_context(tc.tile_pool(name="lnab", bufs=2))
            self.wotp = ctx.enter_context(tc.tile_pool(name="wot", bufs=2))
            self.kep = ctx.enter_context(tc.tile_pool(name="kep", bufs=2))
            self.kvp = ctx.enter_context(tc.tile_pool(name="kvp", bufs=3))
            self.smallp = ctx.enter_context(tc.tile_pool(name="smallp", bufs=4))
            self.recb = ctx.enter_context(tc.tile_pool(name="recb", bufs=4))
            self.atmp = ctx.enter_context(tc.tile_pool(name="atmp", bufs=3))
            self.outcp = ctx.enter_context(tc.tile_pool(name="outcp", bufs=5))
            dram = ctx.enter_context(tc.tile_pool(name="dram", bufs=1,
                                                  space="DRAM"))
            self.p_mm = ctx.enter_context(
                tc.tile_pool(name="p_mm", bufs=2, space="PSUM"))
            self.p_stats = ctx.enter_context(
                tc.tile_pool(name="p_stats", bufs=2, space="PSUM"))
            self.p_bc = ctx.enter_context(
                tc.tile_pool(name="p_bc", bufs=2, space="PSUM"))

            # ---------------- constants ----------------
            self.EPS = const.tile([1, 1], F32, tag="eps", name="eps")
            nc.vector.memset(self.EPS, LN_EPS)
            self.ONESC = const.tile([1, 128], BF16, tag="onesc", name="onesc")
            nc.sync.dma_start(out=self.ONESC, in_=I["ones_c"])
            self.ONESR = const.tile([128, 1], BF16, tag="onesr", name="onesr")
            nc.sync.dma_start(out=self.ONESR, in_=I["ones_r"])
            self.SEL24 = []
            for c in range(FT):
                s = const.tile([72, 128], BF16, tag=f"sel{c}", name=f"sel{c}")
                nc.sync.dma_start(out=s, in_=I["sel24"][c])
                self.SEL24.append(s)

            def cols_tile(name, nt):
                t = const.tile([128, nt], F32, tag=name)
                nc.sync.dma_start(out=t, in_=I[name])
                return t

            BQ = cols_tile("bq_c", FT)
            BO = cols_tile("bo_c", FT)
            SAQ = cols_tile("saq_c", FT)
            SAO = cols_tile("sao_c", FT)
            F1B1 = cols_tile("f1b1_c", IT)
            F1B2 = cols_tile("f1b2_c", FT)
            F2B1 = cols_tile("f2b1_c", IT)
            F2B2 = cols_tile("f2b2_c", FT)

            def bc_tile(apslice, tag):
                t = const.tile([128, C], BF16, tag=tag)
                nc.sync.dma_start(out=t, in_=apslice)
                return t

            EXPBB = [bc_tile(I["expbB"][i], f"expbB{i}") for i in range(NIN)]
            BVB = [bc_tile(I["bvB"][i], f"bvB{i}") for i in range(NIN)]
            SAEXPBB = bc_tile(I["saexpbB"], "saexpbB")
            SABVB = bc_tile(I["sabvB"], "sabvB")

            # ---------------- attn weights (scoped; freed before FFN2) ----
            wpool_ctx = tc.tile_pool(name="weights", bufs=1)
            wpool = wpool_ctx.__enter__()
            WQ = self.load_w512(I["wq"], wpool, "wq")
            WO = self.load_w512(I["wo"], wpool, "wo")
            SWQ = self.load_w512(I["saq"], wpool, "saq")
            SWK = self.load_w512(I["sak"], wpool, "sak")
            SWV = self.load_w512(I["sav"], wpool, "sav")
            SWO = self.load_w512(I["sao"], wpool, "sao")
            WK = [self.load_w512(I["wk"][i], wpool, f"wk{i}") for i in range(NIN)]
            WV = [self.load_w512(I["wv"][i], wpool, f"wv{i}") for i in range(NIN)]

            # ---------------- residual load ----------------
            X = [[self.resid.tile([128, CHUNK], BF16, tag="resid", name="resid")
                  for _ in range(FT)] for _ in range(NCH)]
            for ch in range(NCH):
                for c in range(FT):
                    nc.sync.dma_start(
                        out=X[ch][c],
                        in_=I["xT"][128 * c : 128 * (c + 1),
                                    CHUNK * ch : CHUNK * (ch + 1)])

            cc_in = dram.tile([NIN, 129, C], F32, tag="cc_ca_in", name="cc_ca_in")
            cc_out = dram.tile([NIN, 129, C], F32, tag="cc_ca_out", name="cc_ca_out")
            cc2_in = dram.tile([129, C], F32, tag="cc_sa_in", name="cc_sa_in")
            cc2_out = dram.tile([129, C], F32, tag="cc_sa_out", name="cc_sa_out")

            # ============ phase A: CA kv side ============
            with tc.tile_pool(name="ysp", bufs=10) as ysp, \
                 tc.tile_pool(name="ctxsb", bufs=1) as ctxsbp, \
                 tc.tile_pool(name="p_ctx", bufs=1, space="PSUM") as p_ctx:
                KSA = [ctxsbp.tile([1, C], F32, tag=f"ksacc{i}",
                                   name=f"ksacc{i}") for i in range(NIN)]
                CTX_PS = [p_ctx.tile([128, C], F32, tag=f"ctx{i}", name=f"ctx{i}")
                          for i in range(NIN)]
                for i in range(NIN):
                    YS = {}
                    for ch in range(NCH):
                        yt = []
                        for c in range(FT):
                            y = ysp.tile([128, CHUNK], BF16, tag="ys", name="ys")
                            nc.sync.dma_start(
                                out=y,
                                in_=I["ysT"][i, 128 * c : 128 * (c + 1),
                                             CHUNK * ch : CHUNK * (ch + 1)])
                            yt.append(y)
                        YS[ch] = self.ln_batch({ch: yt}, chunks=[ch])[ch]
                    self.kv_side(YS, WK[i], WV[i], EXPBB[i], BVB[i],
                                 CTX_PS[i], KSA[i])
                for i in range(NIN):
                    ct = ctxsbp.tile([128, C], F32, tag=f"ctc{i}", name=f"ctc{i}")
                    nc.vector.tensor_copy(ct, CTX_PS[i])
                    nc.sync.dma_start(out=cc_in[i, 0:128, :], in_=ct)
                    nc.sync.dma_start(out=cc_in[i, 128 : 129, :], in_=KSA[i])
            nc.gpsimd.collective_compute(
                "AllReduce", OP.add, replica_groups=GROUPS,
                ins=[cc_in[:].opt()], outs=[cc_out[:].opt()])

            # ============ q side (overlaps the AllReduce) ============
            with tc.tile_pool(name="E1", bufs=16) as epool1:
                XS1 = self.ln_batch(X)
                E = self.front_E(XS1, WQ, BQ, epool1)

                # ============ CA back + FFN1 ============
                with tc.tile_pool(name="stag1", bufs=1) as staging1:
                    staged, sgt = self.build_staged(cc_out, NIN, staging1)
                    X1 = self.attn_back(X, E, staged, sgt, NIN, WO, BO,
                                        self.resid)
            X2 = self.ffn(X1, "f1w1", "f1w2", F1B1, F1B2, self.resid)

            # ============ phase C: SA (shared ln4) ============
            XS4 = self.ln_batch(X2)
            with tc.tile_pool(name="ctxsb2", bufs=1) as ctxsbp2, \
                 tc.tile_pool(name="p_ctx2", bufs=1, space="PSUM") as p_ctx2:
                KSA2 = ctxsbp2.tile([1, C], F32, tag="ksacc2", name="ksacc2")
                ctx2_ps = p_ctx2.tile([128, C], F32, tag="ctx2", name="ctx2")
                self.kv_side(XS4, SWK, SWV, SAEXPBB, SABVB, ctx2_ps, KSA2)
                ct2 = ctxsbp2.tile([128, C], F32, tag="ctc2", name="ctc2")
                nc.vector.tensor_copy(ct2, ctx2_ps)
                nc.sync.dma_start(out=cc2_in[0:128, :], in_=ct2)
                nc.sync.dma_start(out=cc2_in[128 : 129, :], in_=KSA2)
            nc.gpsimd.collective_compute(
                "AllReduce", OP.add, replica_groups=GROUPS,
                ins=[cc2_in[:].opt()], outs=[cc2_out[:].opt()])

            with tc.tile_pool(name="E2", bufs=16) as epool2:
                E2 = self.front_E(XS4, SWQ, SAQ, epool2)
                with tc.tile_pool(name="stag2", bufs=1) as staging2:
                    staged2, sgt2 = self.build_staged(cc2_out, 1, staging2)
                    X3 = self.attn_back(X2, E2, staged2, sgt2, 1, SWO, SAO,
                                        self.resid)

            wpool_ctx.__exit__(None, None, None)

            # ============ FFN 2 (fp32 residual out) ============
            def emit_out(ch, m, xo):
                nc.sync.dma_start(
                    out=self.out_t[128 * m : 128 * (m + 1),
                                   CHUNK * ch : CHUNK * (ch + 1)],
                    in_=xo)

            with tc.tile_pool(name="fout", bufs=5) as fout:
                self.ffn(X3, "f2w1", "f2w2", F2B1, F2B2, fout,
                         out_dtype=F32, out_cb=emit_out)


# ---------------------------------------------------------------------------
# host side
# ---------------------------------------------------------------------------
_PROGRAM = None
LAST_RESULTS = None


def _cols(v, nt):
    return np.ascontiguousarray(np.asarray(v, np.float32).reshape(nt, 128).T)


def _host_consts():
    import ml_dtypes
    bf = ml_dtypes.bfloat16
    sgbase = np.zeros((FT, 128, 72), np.float32)
    sel24 = np.zeros((FT, 72, 128), np.float32)
    for c in range(FT):
        for p in range(128):
            h = 2 * c + (1 if p >= 64 else 0)
            sgbase[c, p, h] = 1.0
            # pad columns replicate the S one-hot so the batched
            # reciprocal never sees zeros in unused group rows
            for col in range(72):
                if col % 32 >= 8:
                    sgbase[c, p, col] = 1.0 if col % 8 == h else 0.0
            for j in range(3):
                sel24[c, j * 32 + h, p] = 1.0
    return {
        "ones_c": np.ones((1, 128), bf),
        "ones_r": np.ones((128, 1), bf),
        "sgbase": sgbase.astype(bf),
        "sel24": sel24.astype(bf),
    }


def _make_in_maps(inputs):
    import ml_dtypes
    bf = ml_dtypes.bfloat16
    f = lambda k: np.asarray(inputs[k], np.float32)

    def fold_w(w, g):
        # upload layout W.T [in, out], rows scaled by LN gain
        return np.ascontiguousarray(w.T * g[:, None]).astype(bf)

    def fold_bc(w, b_ln, bias, nt):
        # folded output bias cb = b_ln @ W.T + bias, as column tiles
        return _cols(b_ln @ w.T + bias, nt)

    g1, b1 = f("ln1_g"), f("ln1_b")
    g2, b2 = f("ln2_g"), f("ln2_b")
    g3, b3 = f("ln3_g"), f("ln3_b")
    g4, b4 = f("ln4_g"), f("ln4_b")
    g5, b5 = f("ln5_g"), f("ln5_b")

    wk = f("ca_wk")
    wv = f("ca_wv")
    shared = {
        "wq": fold_w(f("ca_wq"), g1),
        "bq_c": fold_bc(f("ca_wq"), b1, f("ca_bq"), FT),
        "wo": np.ascontiguousarray(f("ca_wo").T).astype(bf),
        "bo_c": _cols(f("ca_bo"), FT),
        "wk": np.stack([fold_w(wk[i], g2[i]) for i in range(NIN)]),
        "wv": np.stack([fold_w(wv[i], g2[i]) for i in range(NIN)]),
        "expbB": np.stack(
            [np.tile(np.exp(b2[i] @ wk[i].T + f("ca_bk")[i]), (128, 1))
             for i in range(NIN)]).astype(bf),
        "bvB": np.stack(
            [np.tile(b2[i] @ wv[i].T + f("ca_bv")[i], (128, 1))
             for i in range(NIN)]).astype(bf),
        "saq": fold_w(f("sa_wq"), g4),
        "saq_c": fold_bc(f("sa_wq"), b4, f("sa_bq"), FT),
        "sak": fold_w(f("sa_wk"), g4),
        "saexpbB": np.tile(np.exp(b4 @ f("sa_wk").T + f("sa_bk")),
                           (128, 1)).astype(bf),
        "sav": fold_w(f("sa_wv"), g4),
        "sabvB": np.tile(b4 @ f("sa_wv").T + f("sa_bv"),
                         (128, 1)).astype(bf),
        "sao": np.ascontiguousarray(f("sa_wo").T).astype(bf),
        "sao_c": _cols(f("sa_bo"), FT),
        "f1w1": fold_w(f("ffn1_w1"), g3),
        "f1b1_c": fold_bc(f("ffn1_w1"), b3, f("ffn1_b1"), IT),
        "f1w2": np.ascontiguousarray(f("ffn1_w2").T).astype(bf),
        "f1b2_c": _cols(f("ffn1_b2"), FT),
        "f2w1": fold_w(f("ffn2_w1"), g5),
        "f2b1_c": fold_bc(f("ffn2_w1"), b5, f("ffn2_b1"), IT),
        "f2w2": np.ascontiguousarray(f("ffn2_w2").T).astype(bf),
        "f2b2_c": _cols(f("ffn2_b2"), FT),
    }
    shared.update(_host_consts())

    x = f("x")
    ys = f("ys")
    in_maps = []
    for core in range(N_CORES):
        b, half = core // 2, core % 2
        lo, hi = half * NTOK, (half + 1) * NTOK
        m = dict(shared)
        m["xT"] = np.ascontiguousarray(x[b, lo:hi, :].T).astype(bf)
        m["ysT"] = np.ascontiguousarray(
            ys[:, b, lo:hi, :].transpose(0, 2, 1)).astype(bf)
        in_maps.append(m)
    return in_maps


def kernel(**inputs):
    global _PROGRAM, LAST_RESULTS
    if _PROGRAM is None:
        _PROGRAM = _build_program()
    nc = _PROGRAM
    in_maps = _make_in_maps(inputs)

    trace = os.environ.get("BASS_TRACE", "") not in ("", "0")
    res = run_bass_kernel_spmd(nc, in_maps, core_ids=list(range(N_CORES)),
                               trace=trace)
    LAST_RESULTS = res

    out = np.empty((B, T, C), np.float32)
    for core in range(N_CORES):
        b, half = core // 2, core % 2
        lo, hi = half * NTOK, (half + 1) * NTOK
        out[b, lo:hi, :] = res.results[core]["outT"].T
    return out


# revision 11
# speedup vs baseline: 1.8374x; 1.1233x over previous
"""Trainium2 Bass kernel for nn_CrossAttentionBlock (B=4, T=4096, C=512, H=8,
INNER=2048, NIN=2) on 8 NeuronCores.

Sharding: core c handles batch b=c//2, token half h=c%2 (2048 tokens each).
All per-token math is local; the only cross-core coupling is the linear-
attention context (ctx = k^T v, [H,64,64] per batch) and k_sum, reduced with
pair-wise AllReduces (cores 2b and 2b+1).

v2 design (all-bf16 matmul datapath):
- Every matmul runs in bf16 (weights pre-cast host-side, activations written
  as bf16 by the producing Act/DVE/Pool op). PSUM accumulation stays fp32.
  This avoids the throttled fp32_mode=HIGH PE path and halves LDWEIGHTS.
- LayerNorm affine (g, b) is folded into the consuming projection weights on
  the host (rows scaled by g, b folded into the output bias), so on-chip LN
  only computes per-token a=rsqrt(var+eps) and mean, then xs = x*aB - amB.
- Per-token LN stats use fast fp32 reciprocal_approx_fast (never the 3.3us
  fp32r-output reciprocal path); rsqrt = recip(sqrt()).
- Loops are batched by activation function (Sqrt / Exp / Gelu) to minimize
  1.28us ACT_TABLE_LOADs, and chunks pipeline across engines.
- Elementwise work is spread across DVE and the otherwise-idle Pool engine.
- k/v sides run before the q sides so the q projections + exp overlap the
  AllReduce; SG/BD stationaries are built with compute-engine copies from a
  staged bf16 ctx tile instead of many small scatter DMAs.
- All weights live in SBUF for the whole kernel (no re-streaming).
"""
import os
import numpy as np

import concourse.bass as bass
import concourse.tile as tile
from concourse import mybir
from concourse.vector_clock import ScopedClock
from concourse.bass_utils import run_bass_kernel_spmd

F32 = mybir.dt.float32
BF16 = mybir.dt.bfloat16
FP8 = mybir.dt.float8e4
W8SCALE = 64.0
AF = mybir.ActivationFunctionType
OP = mybir.AluOpType

B, T, C, H, D, INNER, NIN = 4, 4096, 512, 8, 64, 2048, 2
N_CORES = 8
NTOK = 2048          # tokens per core
CHUNK = 512          # tokens per chunk
NCH = NTOK // CHUNK  # 4 chunks
FT = C // 128        # 4 feature tiles
IT = INNER // 128    # 16 inner tiles
LN_EPS = 1e-5
GROUPS = [[0, 1], [2, 3], [4, 5], [6, 7]]

_split_counter = [0]


def _split_multi_waits(nc):
    """This walrus build only supports one sync-wait per instruction; move
    extra waits onto same-engine NoOps placed immediately before."""
    for f in nc.m.functions:
        for blk in f.blocks:
            out = []
            changed = False
            for inst in blk.instructions:
                si = inst.sync_info
                if si is not None and si.on_wait and len(si.on_wait) > 1:
                    waits = list(si.on_wait)
                    for w in waits[:-1]:
                        _split_counter[0] += 1
                        nop = mybir.InstNoOp(
                            name=f"I-waitsplit-{_split_counter[0]}", ins=[], outs=[]
                        )
                        nop.engine = inst.engine
                        nop.sync_info = mybir.SyncInfo(on_wait=[w], on_update=[])
                        out.append(nop)
                    si.on_wait = waits[-1:]
                    inst.sync_info = si
                    changed = True
                out.append(inst)
            if changed:
                blk.instructions = out


class _TC(tile.TileContext):
    def _drain_and_barrier(self, tick_clock, wait_clock):
        drain_inst = self.nc.sync.drain()
        wait_clock.add_sem_waits(
            drain_inst.ins, ScopedClock({None: tick_clock.global_clock})
        )
        si = drain_inst.ins.sync_info
        if si is not None and si.on_wait and len(si.on_wait) > 1:
            waits = list(si.on_wait)
            si.on_wait = waits[:1]
            drain_inst.ins.sync_info = si
            for i in range(1, len(waits)):
                extra = self.nc.sync.drain()
                esi = extra.ins.sync_info
                if esi is None:
                    extra.ins.sync_info = mybir.SyncInfo(
                        on_wait=waits[i : i + 1], on_update=[]
                    )
                else:
                    esi.on_wait = waits[i : i + 1]
                    extra.ins.sync_info = esi
        self.nc.all_engine_barrier()
        assert self.sems is not None
        popped = self.nc._tile_sem_poison_stack.pop()
        assert popped is self._sem_poison
        self.nc.clear_and_free_semaphores(list(self.sems.allocated().values()))
        self.nc.all_engine_barrier()


def _build_program(split=True):
    nc = bass.Bass("TRN2", target_bir_lowering=False, debug=False, num_devices=N_CORES)
    I = {}

    def di(name, shape, dt=BF16):
        I[name] = nc.dram_tensor(name, list(shape), dt, kind="ExternalInput").ap()

    di("xT", [C, NTOK])
    di("ysT", [NIN, C, NTOK])
    for w in ["wq", "wo", "saq", "sak", "sav", "sao"]:
        di(w, [C, C])
    di("wk", [NIN, C, C])
    di("wv", [NIN, C, C])
    di("f1w1", [FT // 2, 128, 2, INNER], FP8)
    di("f1w2", [IT // 2, 128, 2, C], FP8)
    di("f2w1", [FT // 2, 128, 2, INNER], FP8)
    di("f2w2", [IT // 2, 128, 2, C], FP8)
    for bname in ["bq_c", "bo_c", "saq_c", "sao_c", "f1b2_c", "f2b2_c"]:
        di(bname, [128, FT], F32)
    di("f1b1_c", [128, IT], F32)
    di("f2b1_c", [128, IT], F32)
    di("expbB", [NIN, 128, C])
    di("bvB", [NIN, 128, C])
    di("saexpbB", [128, C])
    di("sabvB", [128, C])
    di("ones_c", [1, 128])
    di("ones_r", [128, 1])
    di("sgbase", [FT, 128, 72])
    di("sel24", [FT, 72, 128])

    out_t = nc.dram_tensor("outT", [C, NTOK], F32, kind="ExternalOutput").ap()

    with _TC(nc) as tc:
        with nc.allow_low_precision(reason="bf16 datapath, tolerance 2e-2"):
            _Emitter(nc, tc, I, out_t).run()
    if split:
        _split_multi_waits(nc)
    return nc


class _Emitter:
    def __init__(self, nc, tc, I, out_t):
        self.nc, self.tc, self.I, self.out_t = nc, tc, I, out_t

    # ---------------- layer norm (folded affine) ----------------
    def act_direct(self, out, in_, func, bias=0.0, scale=1.0):
        """Emit an InstActivation with a func the wrapper refuses
        (Rsqrt/Reciprocal). Accuracy is table-interp grade (~1e-3), fine
        for the 2e-2 gate. Walrus inserts the table load from the final
        func, so mutating post-emission is safe."""
        inst = self.nc.scalar.activation(out=out, in_=in_, func=AF.Sqrt,
                                         bias=bias, scale=scale)
        inst.ins.func = func
        return inst

    def ln_stats(self, x):
        """Stat matmuls + squares for one chunk -> (s_ps, q_ps)."""
        nc = self.nc
        s_ps = self.p_stats.tile([1, CHUNK], F32, tag="stats", name="stats")
        for k in range(FT):
            nc.tensor.matmul(s_ps, self.ONESR, x[k],
                             start=(k == 0), stop=(k == FT - 1))
        xsq = []
        for k in range(FT):
            sq = self.lntmp.tile([128, CHUNK], BF16, tag="xsq", name="xsq")
            nc.scalar.activation(out=sq, in_=x[k], func=AF.Square)
            xsq.append(sq)
        q_ps = self.p_stats.tile([1, CHUNK], F32, tag="stats", name="stats")
        for k in range(FT):
            nc.tensor.matmul(q_ps, self.ONESR, xsq[k],
                             start=(k == 0), stop=(k == FT - 1))
        return s_ps, q_ps

    def ln_finish(self, x, s_ps, q_ps, pairs_pool=None):
        """Row chain + broadcasts + xs tiles for one chunk."""
        nc = self.nc
        mrow = self.rows.tile([1, CHUNK], F32, tag="rows", name="rows")
        nc.vector.tensor_scalar(out=mrow, in0=s_ps, scalar1=1.0 / C,
                                scalar2=None, op0=OP.mult)
        s2 = self.rows.tile([1, CHUNK], F32, tag="rows", name="rows")
        nc.vector.tensor_tensor(out=s2, in0=mrow, in1=mrow, op=OP.mult)
        v = self.rows.tile([1, CHUNK], F32, tag="rows", name="rows")
        nc.vector.scalar_tensor_tensor(out=v, in0=q_ps, scalar=1.0 / C,
                                       in1=s2, op0=OP.mult, op1=OP.subtract)
        arow = self.rowsb.tile([1, CHUNK], BF16, tag="rowsb", name="rowsb")
        self.act_direct(arow, v, AF.Rsqrt, bias=self.EPS, scale=1.0)
        am = self.rowsb.tile([1, CHUNK], BF16, tag="rowsb", name="rowsb")
        nc.vector.tensor_tensor(out=am, in0=arow, in1=mrow, op=OP.mult)
        bc_ps = self.p_bc.tile([128, CHUNK], F32, tag="bc", name="bc")
        nc.tensor.matmul(bc_ps, self.ONESC, arow, start=True, stop=True)
        bc_ps2 = self.p_bc.tile([128, CHUNK], F32, tag="bc", name="bc")
        nc.tensor.matmul(bc_ps2, self.ONESC, am, start=True, stop=True)
        aB = self.lnab.tile([128, CHUNK], BF16, tag="aB", name="aB")
        nc.scalar.activation(out=aB, in_=bc_ps, func=AF.Copy, bias=0.0,
                             scale=1.0)
        amB = self.lnab.tile([128, CHUNK], BF16, tag="amB", name="amB")
        nc.vector.tensor_copy(amB, bc_ps2)
        if pairs_pool is not None:
            # fp8 DoubleRow layout: xs as [128, 2, CHUNK] pair tiles
            xs = []
            for P in range(FT // 2):
                xp = pairs_pool.tile([128, 2, CHUNK], FP8, tag="xp", name="xp")
                for i in range(2):
                    t1 = self.lntmp.tile([128, CHUNK], BF16, tag="lnt", name="lnt")
                    nc.vector.tensor_tensor(out=t1, in0=x[2 * P + i], in1=aB,
                                            op=OP.mult)
                    nc.gpsimd.tensor_tensor(out=xp[:, i, :], in0=t1, in1=amB,
                                            op=OP.subtract)
                xs.append(xp)
            return xs
        xs = []
        for k in range(FT):
            t1 = self.lntmp.tile([128, CHUNK], BF16, tag="lnt", name="lnt")
            nc.vector.tensor_tensor(out=t1, in0=x[k], in1=aB, op=OP.mult)
            xk = self.xnp.tile([128, CHUNK], BF16, tag="xn", name="xn")
            nc.gpsimd.tensor_tensor(out=xk, in0=t1, in1=amB, op=OP.subtract)
            xs.append(xk)
        return xs

    def ln_batch(self, Xin, chunks=None, pairs_pool=None):
        """LN with one-chunk lookahead: chunk N+1's stat matmuls are
        emitted before chunk N's broadcasts so the PE never stalls on the
        row chain."""
        if chunks is None:
            chunks = range(NCH)
        XS = {}
        pend = None
        for ch in chunks:
            st = self.ln_stats(Xin[ch])
            if pend is not None:
                pch, px, ps, pq = pend
                XS[pch] = self.ln_finish(px, ps, pq, pairs_pool)
            pend = (ch, Xin[ch], st[0], st[1])
        pch, px, ps, pq = pend
        XS[pch] = self.ln_finish(px, ps, pq, pairs_pool)
        return XS

    # ---------------- matmul helpers ----------------
    def proj_fm_psum(self, w_tiles, xs, m):
        ps = self.p_mm.tile([128, CHUNK], F32, tag="mm", name="mm")
        for k in range(FT):
            self.nc.tensor.matmul(ps, w_tiles[k][:, 128 * m : 128 * (m + 1)],
                                  xs[k], start=(k == 0), stop=(k == FT - 1))
        return ps

    def proj_tm_psum(self, w_tiles, xs, t):
        ps = self.p_mm.tile([128, CHUNK], F32, tag="mm", name="mm")
        for k in range(FT):
            self.nc.tensor.matmul(ps, xs[k][:, 128 * t : 128 * (t + 1)],
                                  w_tiles[k], start=(k == 0),
                                  stop=(k == FT - 1))
        return ps

    def load_w512(self, ap, pool, tag):
        tiles = []
        for k in range(FT):
            t = pool.tile([128, C], BF16, tag=f"{tag}{k}", name=f"{tag}{k}")
            self.nc.sync.dma_start(out=t, in_=ap[128 * k : 128 * (k + 1), :])
            tiles.append(t)
        return tiles

    # ---------------- attention pieces ----------------
    def kv_side(self, xs_by_ch, WK, WV, expbB, bvB, ctx_ps, ks_acc):
        """k/v projections (token-major, bias via broadcast consts),
        k-softmax over d, head-pair ctx accumulation ([128,C] psum in raw
        pair layout), ks accumulated into SBUF."""
        nc = self.nc
        first = [True]
        for ch in range(NCH):
            xs = xs_by_ch[ch]
            ks_ps = self.p_stats.tile([1, C], F32, tag="stats", name="stats")
            for t in range(FT):
                kps = self.proj_tm_psum(WK, xs, t)
                kE = self.kep.tile([128, C], BF16, tag="kE", name="kE")
                nc.scalar.activation(out=kE, in_=kps, func=AF.Exp)
                kEb = self.kep.tile([128, C], BF16, tag="kEb", name="kEb")
                nc.vector.tensor_tensor(out=kEb, in0=kE, in1=expbB, op=OP.mult)
                ssum = self.smallp.tile([128, H], F32, tag="ssum", name="ssum")
                nc.vector.tensor_reduce(
                    out=ssum, in_=kEb.rearrange("p (h d) -> p h d", d=D),
                    axis=mybir.AxisListType.X, op=OP.add)
                rsum = self.smallp.tile([128, H], F32, tag="rsum", name="rsum")
                nc.vector.reciprocal(out=rsum, in_=ssum)
                kn = self.kvp.tile([128, C], BF16, tag="kn", name="kn")
                for h in range(H):
                    nc.vector.tensor_scalar(
                        out=kn[:, D * h : D * (h + 1)],
                        in0=kEb[:, D * h : D * (h + 1)],
                        scalar1=rsum[:, h : h + 1], scalar2=None, op0=OP.mult)
                vps = self.proj_tm_psum(WV, xs, t)
                vn = self.kvp.tile([128, C], BF16, tag="vn", name="vn")
                nc.vector.tensor_tensor(out=vn, in0=vps, in1=bvB, op=OP.add)
                for j in range(FT):
                    nc.tensor.matmul(
                        ctx_ps[:, 128 * j : 128 * (j + 1)],
                        kn[:, 128 * j : 128 * (j + 1)],
                        vn[:, 128 * j : 128 * (j + 1)],
                        start=(first[0] and j == 0),
                        stop=(ch == NCH - 1 and t == FT - 1 and j == FT - 1))
                nc.tensor.matmul(ks_ps, self.ONESR, kn, start=(t == 0),
                                 stop=(t == FT - 1))
                first[0] = False
            if ch == 0:
                nc.vector.tensor_copy(ks_acc, ks_ps)
            else:
                nc.vector.tensor_tensor(out=ks_acc, in0=ks_acc, in1=ks_ps,
                                        op=OP.add)

    def front_E(self, xs_by_ch, W, b_cols, epool):
        """q projection + exp -> E tiles (bf16). Act func: Exp."""
        nc = self.nc
        E = [[None] * FT for _ in range(NCH)]
        for ch in range(NCH):
            for m in range(FT):
                ps = self.proj_fm_psum(W, xs_by_ch[ch], m)
                e = epool.tile([128, CHUNK], BF16, tag="E", name="E")
                nc.scalar.activation(out=e, in_=ps, func=AF.Exp,
                                     bias=b_cols[:, m : m + 1], scale=1.0)
                E[ch][m] = e
        return E

    def build_staged(self, cc_ap, n_in, staging):
        """From AllReduced cc ([n,129,C] or [129,C] fp32 DRAM): per input a
        [128,C] bf16 tile in the raw head-pair layout (even heads on
        partitions 0:63, odd on 64:127), a transposed ksum [128,FT] fp32
        tile, and sg stationaries [128,ncols] bf16 per feature tile."""
        nc, I = self.nc, self.I
        cc = (lambda i: cc_ap[i]) if n_in > 1 else (lambda i: cc_ap)
        ncols = 32 * n_in + 8
        staged, sgt = [], []
        for c in range(FT):
            sg = staging.tile([128, ncols], BF16, tag=f"sg{c}", name=f"sg{c}")
            nc.sync.dma_start(out=sg, in_=I["sgbase"][c][:, 0:ncols])
            sgt.append(sg)
        for i in range(n_in):
            stb = staging.tile([128, C], BF16, tag=f"stb{i}", name=f"stb{i}")
            nc.sync.dma_start(out=stb, in_=cc(i)[0:128, :])
            kst = staging.tile([128, FT], BF16, tag=f"kst{i}", name=f"kst{i}")
            nc.sync.dma_start(
                out=kst,
                in_=cc(i)[128 : 129, :].rearrange("o (a p) -> p (o a)", p=128))
            for c in range(FT):
                col = 32 * (1 + i) + 2 * c
                nc.scalar.activation(
                    out=sgt[c][0:D, col : col + 1], in_=kst[0:D, c : c + 1],
                    func=AF.Copy, bias=0.0, scale=1.0)
                nc.scalar.activation(
                    out=sgt[c][D:128, col + 1 : col + 2],
                    in_=kst[D:128, c : c + 1],
                    func=AF.Copy, bias=0.0, scale=1.0)
            staged.append(stb)
        return staged, sgt

    def attn_back(self, Xin, E, staged, sgt, n_in, WO, bo_cols, out_pool):
        """Denominators (single M=8+8n SG matmul), reciprocal, broadcasts,
        per-head ctx apply (split-K from staged bf16), assembly, wo
        projection + residual."""
        nc, tc = self.nc, self.tc
        ncols = 32 * n_in + 8
        Xout = [[None] * FT for _ in range(NCH)]
        # --- all denominators + reciprocals (no act tables) ---
        RR = []
        p_sg_ctx = tc.tile_pool(name="p_sg", bufs=2, space="PSUM")
        p_sg = p_sg_ctx.__enter__()
        for ch in range(NCH):
            g_ps = p_sg.tile([ncols, CHUNK], F32, tag="sg", name="sg")
            for c in range(FT):
                nc.tensor.matmul(g_ps, sgt[c][:, 0:ncols], E[ch][c],
                                 start=(c == 0), stop=(c == FT - 1))
            rr = self.recb.tile([ncols, CHUNK], BF16, tag="recb", name="recb")
            self.act_direct(rr, g_ps, AF.Reciprocal)
            RR.append(rr)
        p_sg_ctx.__exit__(None, None, None)
        # --- apply + assemble + project ---
        for ch in range(NCH):
            rr = RR[ch]
            outc = []
            for c in range(FT):
                sb_ps = self.p_bc.tile([128, CHUNK], F32, tag="bc", name="bc")
                nc.tensor.matmul(sb_ps, self.SEL24[c][0:8, :], rr[0:8, :],
                                 start=True, stop=True)
                ssb = self.atmp.tile([128, CHUNK], BF16, tag="gbs", name="gbs")
                nc.scalar.activation(out=ssb, in_=sb_ps, func=AF.Copy,
                                     bias=0.0, scale=1.0)
                acc = self.atmp.tile([128, CHUNK], BF16, tag="asm", name="asm")
                nc.vector.tensor_tensor(out=acc, in0=E[ch][c], in1=ssb,
                                        op=OP.mult)
                for i in range(n_in):
                    a_ps = self.p_mm.tile([128, CHUNK], F32, tag="mm", name="mm")
                    nc.tensor.matmul(
                        a_ps[0:D, :],
                        staged[i][0:D, 128 * c : 128 * c + D],
                        E[ch][c][0:D, :], start=True, stop=True)
                    nc.tensor.matmul(
                        a_ps[D:128, :],
                        staged[i][D:128, 128 * c + D : 128 * (c + 1)],
                        E[ch][c][D:128, :], start=True, stop=True)
                    j = 32 * (1 + i)
                    gb_ps = self.p_bc.tile([128, CHUNK], F32, tag="bc", name="bc")
                    nc.tensor.matmul(gb_ps, self.SEL24[c][j : j + 8, :],
                                     rr[j : j + 8, :],
                                     start=True, stop=True)
                    gsb = self.atmp.tile([128, CHUNK], BF16, tag="gbs", name="gbs")
                    nc.scalar.activation(out=gsb, in_=gb_ps, func=AF.Copy,
                                         bias=0.0, scale=1.0)
                    ai = self.atmp.tile([128, CHUNK], BF16, tag="asm", name="asm")
                    nc.vector.tensor_tensor(out=ai, in0=a_ps, in1=gsb,
                                            op=OP.mult)
                    nxt = self.outcp.tile([128, CHUNK], BF16, tag="outc", name="outc") \
                        if i == n_in - 1 else \
                        self.atmp.tile([128, CHUNK], BF16, tag="asm", name="asm")
                    nc.vector.tensor_tensor(out=nxt, in0=acc, in1=ai, op=OP.add)
                    acc = nxt
                outc.append(acc)
            for m in range(FT):
                wps = self.proj_fm_psum(WO, outc, m)
                tt = self.wotp.tile([128, CHUNK], F32, tag="wot", name="wot")
                nc.scalar.activation(out=tt, in_=wps, func=AF.Identity,
                                     bias=bo_cols[:, m : m + 1], scale=1.0)
                xo = out_pool.tile([128, CHUNK], BF16, tag="resid", name="resid")
                nc.gpsimd.tensor_tensor(out=xo, in0=Xin[ch][m], in1=tt,
                                        op=OP.add)
                Xout[ch][m] = xo
        return Xout

    def ffn(self, Xin, w1name, w2name, B1, B2, out_pool, out_dtype=BF16,
            out_cb=None):
        """fp8 DoubleRow FFN: weights pre-scaled by W8SCALE host-side and
        laid out as [128, 2, f] contraction pairs; the 1/W8SCALE unscale is
        folded into the Act scale. xs and h live in fp8 pair tiles."""
        nc, tc, I = self.nc, self.tc, self.I
        DR = mybir.MatmulPerfMode.DoubleRow
        Xout = [[None] * FT for _ in range(NCH)]
        with tc.tile_pool(name=w1name, bufs=1) as wp, \
             tc.tile_pool(name="hp", bufs=IT // 2) as hp, \
             tc.tile_pool(name="xp8", bufs=6) as xp8, \
             tc.tile_pool(name="p_acc", bufs=2, space="PSUM") as p_acc:
            W1 = []
            for P in range(FT // 2):
                t = wp.tile([128, 2, INNER], FP8, tag=f"w1_{P}", name=f"w1_{P}")
                nc.sync.dma_start(out=t, in_=I[w1name][P])
                W1.append(t)
            W2 = []
            for K in range(IT // 2):
                t = wp.tile([128, 2, C], FP8, tag=f"w2_{K}", name=f"w2_{K}")
                nc.sync.dma_start(out=t, in_=I[w2name][K])
                W2.append(t)
            XS = self.ln_batch(Xin, pairs_pool=xp8)
            for ch in range(NCH):
                xs = XS[ch]
                hs = []
                for k in range(IT):
                    hps = self.p_mm.tile([128, CHUNK], F32, tag="mm", name="mm")
                    for P in range(FT // 2):
                        nc.tensor.matmul(
                            hps, W1[P][:, :, 128 * k : 128 * (k + 1)], xs[P],
                            start=(P == 0), stop=(P == FT // 2 - 1),
                            perf_mode=DR)
                    if k % 2 == 0:
                        hpair = hp.tile([128, 2, CHUNK], FP8, tag="h", name="h")
                        hs.append(hpair)
                    nc.scalar.activation(out=hs[k // 2][:, k % 2, :], in_=hps,
                                         func=AF.Gelu_apprx_tanh,
                                         bias=B1[:, k : k + 1],
                                         scale=1.0 / W8SCALE)
                for m in range(FT):
                    acc = p_acc.tile([128, CHUNK], F32, tag="acc", name="acc")
                    for K in range(IT // 2):
                        nc.tensor.matmul(
                            acc, W2[K][:, :, 128 * m : 128 * (m + 1)], hs[K],
                            start=(K == 0), stop=(K == IT // 2 - 1),
                            perf_mode=DR)
                    tt = self.wotp.tile([128, CHUNK], F32, tag="wot", name="wot")
                    nc.scalar.activation(out=tt, in_=acc, func=AF.Identity,
                                         bias=B2[:, m : m + 1],
                                         scale=1.0 / W8SCALE)
                    xo = out_pool.tile([128, CHUNK], out_dtype, tag="resid",
                                       name="resid")
                    nc.gpsimd.tensor_tensor(out=xo, in0=Xin[ch][m], in1=tt,
                                            op=OP.add)
                    Xout[ch][m] = xo
                    if out_cb is not None:
                        out_cb(ch, m, xo)
        return Xout

    # ---------------- main ----------------
    def run(self):
        nc, tc, I = self.nc, self.tc, self.I
        from contextlib import ExitStack

        with ExitStack() as ctx:
            const = ctx.enter_context(tc.tile_pool(name="const", bufs=1))
            self.resid = ctx.enter_context(tc.tile_pool(name="resid", bufs=36))
            self.xnp = ctx.enter_context(tc.tile_pool(name="xn", bufs=18))
            self.rows = ctx.enter_context(tc.tile_pool(name="rows", bufs=4))
            self.rowsb = ctx.enter_context(tc.tile_pool(name="rowsb", bufs=4))
            self.lntmp = ctx.enter_context(tc.tile_pool(name="lntmp", bufs=4))
            self.lnab = ctx.enter_context(tc.tile_pool(name="lnab", bufs=2))
            self.wotp = ctx.enter_context(tc.tile_pool(name="wot", bufs=2))
            self.kep = ctx.enter_context(tc.tile_pool(name="kep", bufs=2))
            self.kvp = ctx.enter_context(tc.tile_pool(name="kvp", bufs=3))
            self.smallp = ctx.enter_context(tc.tile_pool(name="smallp", bufs=4))
            self.recb = ctx.enter_context(tc.tile_pool(name="recb", bufs=4))
            self.atmp = ctx.enter_context(tc.tile_pool(name="atmp", bufs=3))
            self.outcp = ctx.enter_context(tc.tile_pool(name="outcp", bufs=5))
            dram = ctx.enter_context(tc.tile_pool(name="dram", bufs=1,
                                                  space="DRAM"))
            self.p_mm = ctx.enter_context(
                tc.tile_pool(name="p_mm", bufs=2, space="PSUM"))
            self.p_stats = ctx.enter_context(
                tc.tile_pool(name="p_stats", bufs=2, space="PSUM"))
            self.p_bc = ctx.enter_context(
                tc.tile_pool(name="p_bc", bufs=2, space="PSUM"))

            # ---------------- constants ----------------
            self.EPS = const.tile([1, 1], F32, tag="eps", name="eps")
            nc.vector.memset(self.EPS, LN_EPS)
            self.ONESC = const.tile([1, 128], BF16, tag="onesc", name="onesc")
            nc.sync.dma_start(out=self.ONESC, in_=I["ones_c"])
            self.ONESR = const.tile([128, 1], BF16, tag="onesr", name="onesr")
            nc.sync.dma_start(out=self.ONESR, in_=I["ones_r"])
            self.SEL24 = []
            for c in range(FT):
                s = const.tile([72, 128], BF16, tag=f"sel{c}", name=f"sel{c}")
                nc.sync.dma_start(out=s, in_=I["sel24"][c])
                self.SEL24.append(s)

            def cols_tile(name, nt):
                t = const.tile([128, nt], F32, tag=name)
                nc.sync.dma_start(out=t, in_=I[name])
                return t

            BQ = cols_tile("bq_c", FT)
            BO = cols_tile("bo_c", FT)
            SAQ = cols_tile("saq_c", FT)
            SAO = cols_tile("sao_c", FT)
            F1B1 = cols_tile("f1b1_c", IT)
            F1B2 = cols_tile("f1b2_c", FT)
            F2B1 = cols_tile("f2b1_c", IT)
            F2B2 = cols_tile("f2b2_c", FT)

            def bc_tile(apslice, tag):
                t = const.tile([128, C], BF16, tag=tag)
                nc.sync.dma_start(out=t, in_=apslice)
                return t

            EXPBB = [bc_tile(I["expbB"][i], f"expbB{i}") for i in range(NIN)]
            BVB = [bc_tile(I["bvB"][i], f"bvB{i}") for i in range(NIN)]
            SAEXPBB = bc_tile(I["saexpbB"], "saexpbB")
            SABVB = bc_tile(I["sabvB"], "sabvB")

            # ---------------- attn weights (scoped; freed before FFN2) ----
            wpool_ctx = tc.tile_pool(name="weights", bufs=1)
            wpool = wpool_ctx.__enter__()
            WQ = self.load_w512(I["wq"], wpool, "wq")
            WO = self.load_w512(I["wo"], wpool, "wo")
            SWQ = self.load_w512(I["saq"], wpool, "saq")
            SWK = self.load_w512(I["sak"], wpool, "sak")
            SWV = self.load_w512(I["sav"], wpool, "sav")
            SWO = self.load_w512(I["sao"], wpool, "sao")
            WK = [self.load_w512(I["wk"][i], wpool, f"wk{i}") for i in range(NIN)]
            WV = [self.load_w512(I["wv"][i], wpool, f"wv{i}") for i in range(NIN)]

            # ---------------- residual load ----------------
            X = [[self.resid.tile([128, CHUNK], BF16, tag="resid", name="resid")
                  for _ in range(FT)] for _ in range(NCH)]
            for ch in range(NCH):
                for c in range(FT):
                    nc.sync.dma_start(
                        out=X[ch][c],
                        in_=I["xT"][128 * c : 128 * (c + 1),
                                    CHUNK * ch : CHUNK * (ch + 1)])

            cc_in = dram.tile([NIN, 129, C], BF16, tag="cc_ca_in", name="cc_ca_in")
            cc_out = dram.tile([NIN, 129, C], BF16, tag="cc_ca_out", name="cc_ca_out")
            cc2_in = dram.tile([129, C], BF16, tag="cc_sa_in", name="cc_sa_in")
            cc2_out = dram.tile([129, C], BF16, tag="cc_sa_out", name="cc_sa_out")

            # ============ phase A: CA kv side ============
            with tc.tile_pool(name="ysp", bufs=10) as ysp, \
                 tc.tile_pool(name="ctxsb", bufs=1) as ctxsbp, \
                 tc.tile_pool(name="p_ctx", bufs=1, space="PSUM") as p_ctx:
                KSA = [ctxsbp.tile([1, C], F32, tag=f"ksacc{i}",
                                   name=f"ksacc{i}") for i in range(NIN)]
                CTX_PS = [p_ctx.tile([128, C], F32, tag=f"ctx{i}", name=f"ctx{i}")
                          for i in range(NIN)]
                for i in range(NIN):
                    YS = {}
                    for ch in range(NCH):
                        yt = []
                        for c in range(FT):
                            y = ysp.tile([128, CHUNK], BF16, tag="ys", name="ys")
                            nc.sync.dma_start(
                                out=y,
                                in_=I["ysT"][i, 128 * c : 128 * (c + 1),
                                             CHUNK * ch : CHUNK * (ch + 1)])
                            yt.append(y)
                        YS[ch] = self.ln_batch({ch: yt}, chunks=[ch])[ch]
                    self.kv_side(YS, WK[i], WV[i], EXPBB[i], BVB[i],
                                 CTX_PS[i], KSA[i])
                for i in range(NIN):
                    ct = ctxsbp.tile([128, C], BF16, tag=f"ctc{i}", name=f"ctc{i}")
                    nc.vector.tensor_copy(ct, CTX_PS[i])
                    ksb = ctxsbp.tile([1, C], BF16, tag=f"ksb{i}", name=f"ksb{i}")
                    nc.vector.tensor_copy(ksb, KSA[i])
                    nc.sync.dma_start(out=cc_in[i, 0:128, :], in_=ct)
                    nc.sync.dma_start(out=cc_in[i, 128 : 129, :], in_=ksb)
            nc.gpsimd.collective_compute(
                "AllReduce", OP.add, replica_groups=GROUPS,
                ins=[cc_in[:].opt()], outs=[cc_out[:].opt()])

            # ============ q side (overlaps the AllReduce) ============
            with tc.tile_pool(name="E1", bufs=16) as epool1:
                XS1 = self.ln_batch(X)
                E = self.front_E(XS1, WQ, BQ, epool1)

                # ============ CA back + FFN1 ============
                with tc.tile_pool(name="stag1", bufs=1) as staging1:
                    staged, sgt = self.build_staged(cc_out, NIN, staging1)
                    X1 = self.attn_back(X, E, staged, sgt, NIN, WO, BO,
                                        self.resid)
            X2 = self.ffn(X1, "f1w1", "f1w2", F1B1, F1B2, self.resid)

            # ============ phase C: SA (shared ln4) ============
            XS4 = self.ln_batch(X2)
            with tc.tile_pool(name="ctxsb2", bufs=1) as ctxsbp2, \
                 tc.tile_pool(name="p_ctx2", bufs=1, space="PSUM") as p_ctx2:
                KSA2 = ctxsbp2.tile([1, C], F32, tag="ksacc2", name="ksacc2")
                ctx2_ps = p_ctx2.tile([128, C], F32, tag="ctx2", name="ctx2")
                self.kv_side(XS4, SWK, SWV, SAEXPBB, SABVB, ctx2_ps, KSA2)
                ct2 = ctxsbp2.tile([128, C], BF16, tag="ctc2", name="ctc2")
                nc.vector.tensor_copy(ct2, ctx2_ps)
                ksb2 = ctxsbp2.tile([1, C], BF16, tag="ksb2", name="ksb2")
                nc.vector.tensor_copy(ksb2, KSA2)
                nc.sync.dma_start(out=cc2_in[0:128, :], in_=ct2)
                nc.sync.dma_start(out=cc2_in[128 : 129, :], in_=ksb2)
            nc.gpsimd.collective_compute(
                "AllReduce", OP.add, replica_groups=GROUPS,
                ins=[cc2_in[:].opt()], outs=[cc2_out[:].opt()])

            with tc.tile_pool(name="E2", bufs=16) as epool2:
                E2 = self.front_E(XS4, SWQ, SAQ, epool2)
                with tc.tile_pool(name="stag2", bufs=1) as staging2:
                    staged2, sgt2 = self.build_staged(cc2_out, 1, staging2)
                    X3 = self.attn_back(X2, E2, staged2, sgt2, 1, SWO, SAO,
                                        self.resid)

            wpool_ctx.__exit__(None, None, None)

            # ============ FFN 2 (fp32 residual out) ============
            def emit_out(ch, m, xo):
                nc.sync.dma_start(
                    out=self.out_t[128 * m : 128 * (m + 1),
                                   CHUNK * ch : CHUNK * (ch + 1)],
                    in_=xo)

            with tc.tile_pool(name="fout", bufs=5) as fout:
                self.ffn(X3, "f2w1", "f2w2", F2B1, F2B2, fout,
                         out_dtype=F32, out_cb=emit_out)


# ---------------------------------------------------------------------------
# host side
# ---------------------------------------------------------------------------
_PROGRAM = None
LAST_RESULTS = None


def _fp8_pairs(wu):
    """[K, M] (K = contraction) -> [K//256, 128, 2, M] fp8 DoubleRow pairs,
    scaled by W8SCALE and clipped to TRN e4m3 range."""
    import ml_dtypes
    K, M = wu.shape
    w = np.clip(wu * W8SCALE, -240.0, 240.0)
    w = w.reshape(K // 256, 2, 128, M).transpose(0, 2, 1, 3)
    return np.ascontiguousarray(w).astype(ml_dtypes.float8_e4m3fn)


def _cols(v, nt):
    return np.ascontiguousarray(np.asarray(v, np.float32).reshape(nt, 128).T)


def _host_consts():
    import ml_dtypes
    bf = ml_dtypes.bfloat16
    sgbase = np.zeros((FT, 128, 72), np.float32)
    sel24 = np.zeros((FT, 72, 128), np.float32)
    for c in range(FT):
        for p in range(128):
            h = 2 * c + (1 if p >= 64 else 0)
            sgbase[c, p, h] = 1.0
            # pad columns replicate the S one-hot so the batched
            # reciprocal never sees zeros in unused group rows
            for col in range(72):
                if col % 32 >= 8:
                    sgbase[c, p, col] = 1.0 if col % 8 == h else 0.0
            for j in range(3):
                sel24[c, j * 32 + h, p] = 1.0
    return {
        "ones_c": np.ones((1, 128), bf),
        "ones_r": np.ones((128, 1), bf),
        "sgbase": sgbase.astype(bf),
        "sel24": sel24.astype(bf),
    }


def _make_in_maps(inputs):
    import ml_dtypes
    bf = ml_dtypes.bfloat16
    f = lambda k: np.asarray(inputs[k], np.float32)

    def fold_w(w, g):
        # upload layout W.T [in, out], rows scaled by LN gain
        return np.ascontiguousarray(w.T * g[:, None]).astype(bf)

    def fold_bc(w, b_ln, bias, nt):
        # folded output bias cb = b_ln @ W.T + bias, as column tiles
        return _cols(b_ln @ w.T + bias, nt)

    g1, b1 = f("ln1_g"), f("ln1_b")
    g2, b2 = f("ln2_g"), f("ln2_b")
    g3, b3 = f("ln3_g"), f("ln3_b")
    g4, b4 = f("ln4_g"), f("ln4_b")
    g5, b5 = f("ln5_g"), f("ln5_b")

    wk = f("ca_wk")
    wv = f("ca_wv")
    shared = {
        "wq": fold_w(f("ca_wq"), g1),
        "bq_c": fold_bc(f("ca_wq"), b1, f("ca_bq"), FT),
        "wo": np.ascontiguousarray(f("ca_wo").T).astype(bf),
        "bo_c": _cols(f("ca_bo"), FT),
        "wk": np.stack([fold_w(wk[i], g2[i]) for i in range(NIN)]),
        "wv": np.stack([fold_w(wv[i], g2[i]) for i in range(NIN)]),
        "expbB": np.stack(
            [np.tile(np.exp(b2[i] @ wk[i].T + f("ca_bk")[i]), (128, 1))
             for i in range(NIN)]).astype(bf),
        "bvB": np.stack(
            [np.tile(b2[i] @ wv[i].T + f("ca_bv")[i], (128, 1))
             for i in range(NIN)]).astype(bf),
        "saq": fold_w(f("sa_wq"), g4),
        "saq_c": fold_bc(f("sa_wq"), b4, f("sa_bq"), FT),
        "sak": fold_w(f("sa_wk"), g4),
        "saexpbB": np.tile(np.exp(b4 @ f("sa_wk").T + f("sa_bk")),
                           (128, 1)).astype(bf),
        "sav": fold_w(f("sa_wv"), g4),
        "sabvB": np.tile(b4 @ f("sa_wv").T + f("sa_bv"),
                         (128, 1)).astype(bf),
        "sao": np.ascontiguousarray(f("sa_wo").T).astype(bf),
        "sao_c": _cols(f("sa_bo"), FT),
        "f1w1": _fp8_pairs(f("ffn1_w1").T * g3[:, None]),
        "f1b1_c": fold_bc(f("ffn1_w1"), b3, f("ffn1_b1"), IT),
        "f1w2": _fp8_pairs(f("ffn1_w2").T),
        "f1b2_c": _cols(f("ffn1_b2"), FT),
        "f2w1": _fp8_pairs(f("ffn2_w1").T * g5[:, None]),
        "f2b1_c": fold_bc(f("ffn2_w1"), b5, f("ffn2_b1"), IT),
        "f2w2": _fp8_pairs(f("ffn2_w2").T),
        "f2b2_c": _cols(f("ffn2_b2"), FT),
    }
    shared.update(_host_consts())

    x = f("x")
    ys = f("ys")
    in_maps = []
    for core in range(N_CORES):
        b, half = core // 2, core % 2
        lo, hi = half * NTOK, (half + 1) * NTOK
        m = dict(shared)
        m["xT"] = np.ascontiguousarray(x[b, lo:hi, :].T).astype(bf)
        m["ysT"] = np.ascontiguousarray(
            ys[:, b, lo:hi, :].transpose(0, 2, 1)).astype(bf)
        in_maps.append(m)
    return in_maps


def kernel(**inputs):
    global _PROGRAM, LAST_RESULTS
    if _PROGRAM is None:
        _PROGRAM = _build_program()
    nc = _PROGRAM
    in_maps = _make_in_maps(inputs)

    trace = os.environ.get("BASS_TRACE", "") not in ("", "0")
    res = run_bass_kernel_spmd(nc, in_maps, core_ids=list(range(N_CORES)),
                               trace=trace)
    LAST_RESULTS = res

    out = np.empty((B, T, C), np.float32)
    for core in range(N_CORES):
        b, half = core // 2, core % 2
        lo, hi = half * NTOK, (half + 1) * NTOK
        out[b, lo:hi, :] = res.results[core]["outT"].T
    return out
